# revision 45
# baseline (speedup 1.0000x reference)
"""GCN (5-layer ColorGNN) Bass kernel for 8 TRN2 NeuronCores — v2.

Pipelined design (node-sharded SPMD, 6272 padded nodes/core = 49 tiles):

  - Full padded X (fp16, gpid layout) is staged on EVERY core: the layer-1
    aggregation gathers it directly -> no AllGather for layer 1.
  - Aggregation outputs are produced TRANSPOSED (H^T tiles: features on
    partitions, 128 dst nodes on free): out^T[f,n] = sum_e msg[e,f]*S[e,n]
    with lhsT=msg (gathered rows), rhs=S (one-hot x norm). The full
    symmetric norm dinv[src]*ew*dinv[dst] is folded into S on the host, and
    the bias is a per-partition ACT bias in the relu epilogue.
  - H^T tiles feed the next dense matmul as lhsT straight from SBUF: no
    transposes, no H round-trips through DRAM. Dense outputs T (node-major)
    go to DRAM only as AllGather input. Layer-1's dense pair runs in the
    transposed orientation (lhsT = W1 blocks) so the chain stays in H^T.
  - Each T AllGather is split in 2 node-chunks (4096+2176 rows/core,
    matching gpid regions [0,32768) and [32768,50176) which also serve as
    the int16 gather-index regions). Chunk0 fires when dense tiles 0..31
    are stored; the next layer's aggregation runs in 2 passes (region A
    then region B) so its region-A gathers need only chunk0 -> chunk1's
    transfer hides under the region-A gather. Pass A's PSUM is staged to
    DRAM as an fp16 partial and re-added in pass B via an identity matmul.
  - Host preprocessing: edges bucketed per (core=dst core, dst tile, src
    region), padded to 128-multiples of the cross-core max so all 8 cores
    run one SPMD program.
"""

import numpy as np
import concourse.bass as bass
import concourse.mybir as mybir
import concourse.tile as tile

FP16 = mybir.dt.float16
F32 = mybir.dt.float32
I16 = mybir.dt.int16

P = 128
NCORE = 8
PCN = 6250            # real nodes per core
NPC = 6272            # padded nodes per core (49 tiles)
NT = 49
# AllGather chunks per core (tile-aligned): tiles 0-15 / 16-31 / 32-48
CH = [0, 2048, 4096, 6272]       # per-core row boundaries
GB = [0, 16384, 32768, 50176]    # global gpid base of each chunk
REG0 = 32768          # gpid boundary of gather region 0 (chunks 0a+0b)
NPT = NCORE * NPC     # 50176 padded total nodes
T_SPLIT = 32          # first tile of gather-region 1
GMAX = 8              # chunks per gather call (1024 idx ucode cap)
SBATCH = 8            # chunks per batched S-build DVE op

FEAT = 512
WG = [512, 1024, 512, 128, 128]   # gather/agg width per layer l = 0..4
# dense widths: d0: 512->2048(T out), d1: 2048->1024, d2: 1024->512,
#               d3: 512->128, d4: 128->128, d5: 128->4


# ---------------------------------------------------------------- tile patch
def apply_tile_patch():
    """This walrus build allows only 1 sync-wait per Drain; split the tail
    drain's waits across a chain of drains."""
    import bass_rust

    def _drain_and_barrier_split(self, tick_clock, wait_clock):
        from bass_rust import ScopedClock
        drain_inst = self.nc.sync.drain()
        wait_clock.add_sem_waits(
            drain_inst.ins, ScopedClock({None: tick_clock.global_clock})
        )
        si = drain_inst.ins.sync_info
        waits = list(si.on_wait) if si is not None else []
        if len(waits) > 1:
            si.on_wait = [waits[0]]
            for w in waits[1:]:
                extra = self.nc.sync.drain()
                if extra.ins.sync_info is None:
                    extra.ins.sync_info = bass_rust.SyncInfo(
                        on_wait=[w], on_update=[])
                else:
                    extra.ins.sync_info.on_wait = [w]
        self.nc.all_engine_barrier()
        popped = self.nc._tile_sem_poison_stack.pop()
        assert popped is self._sem_poison
        self.nc.clear_and_free_semaphores(list(self.sems.allocated().values()))
        self.nc.all_engine_barrier()

    tile.TileContext._drain_and_barrier = _drain_and_barrier_split


def _gpid(core, slot):
    """Chunk-major padded global id (vectorized over 3 AG chunks)."""
    out = np.empty_like(np.broadcast_arrays(core, slot)[1])
    core = np.asarray(core)
    slot = np.asarray(slot)
    for j in range(3):
        m = (slot >= CH[j]) & (slot < CH[j + 1])
        w = CH[j + 1] - CH[j]
        out[m] = GB[j] + core[m] * w + (slot[m] - CH[j])
    return out


# ------------------------------------------------------------- preprocess
def preprocess(x, edge_index, edge_attr, Ws, bs, Wp, bp):
    """Host-side: normalization, edge sharding/sorting/packing, weight packs.
    Returns (in_maps, meta)."""
    N = NCORE * PCN
    src = np.asarray(edge_index[0], dtype=np.int64)
    dst = np.asarray(edge_index[1], dtype=np.int64)
    ew = np.asarray(edge_attr, dtype=np.float32)
    loop = np.arange(N, dtype=np.int64)
    src2 = np.concatenate([src, loop])
    dst2 = np.concatenate([dst, loop])
    ew2 = np.concatenate([ew, np.ones(N, np.float32)])

    deg = np.bincount(dst2, weights=ew2.astype(np.float64), minlength=N)
    deg = deg.astype(np.float32)
    dinv = np.where(deg > 0, 1.0 / np.sqrt(deg), 0.0).astype(np.float32)
    normf = (dinv[src2] * ew2 * dinv[dst2]).astype(np.float32)  # full norm

    score = src2 // PCN

    # ---- degree-balanced node->tile packing (within each (core, chunk)):
    # re-bin nodes into tiles so per-(tile, region) edge counts are nearly
    # equal across cores and land just under multiples of 128, minimizing
    # SPMD padding. A node keeps its (core, chunk), so src regions are
    # unchanged by the permutation.
    allslot = np.arange(N, dtype=np.int64) % PCN
    chunk_of_node = np.digitize(allslot, CH[1:3])          # [N] in {0,1,2}
    src_chunk = chunk_of_node[src2]
    dvec = np.zeros((N, 3), np.int64)
    np.add.at(dvec, (dst2, src_chunk), 1)

    # per-(core, chunk, r) totals -> global per-(tile-in-chunk, r) caps
    core_all = np.arange(N, dtype=np.int64) // PCN
    Tcr = np.zeros((NCORE, 3, 3), np.int64)
    for r in range(3):
        np.add.at(Tcr, (core_all, chunk_of_node, r), dvec[:, r])
    newslot = np.zeros(N, np.int64)
    for j in range(3):
        nt_j = (CH[j + 1] - CH[j]) // P
        maxT = Tcr[:, j, :].max(axis=0)                    # [3]
        caps = np.zeros((nt_j, 3), np.int64)
        for r in range(3):
            m_tot = int(np.ceil((maxT[r] + 16 * nt_j) / P))
            base, extra = divmod(m_tot, nt_j)
            caps[:, r] = P * base
            caps[:extra, r] += P
        for c in range(NCORE):
            vs = np.where((core_all == c) & (chunk_of_node == j))[0]
            dv = dvec[vs]
            order_v = np.argsort(-dv.sum(axis=1), kind="stable")
            used = np.zeros((nt_j, 3), np.float64)
            cnt = np.zeros(nt_j, np.int64)
            pos = np.zeros(nt_j, np.int64)
            for vi in order_v:
                nd = dv[vi]
                ok = cnt < P
                over = np.maximum(used + nd - caps, 0).sum(axis=1)
                frac = ((used + nd) / caps).max(axis=1)
                tscore = over * 1e9 + frac
                tscore[~ok] = np.inf
                ti = int(tscore.argmin())
                used[ti] += nd
                cnt[ti] += 1
                newslot[vs[vi]] = CH[j] + ti * P + pos[ti]
                pos[ti] += 1

    core_of = dst2 // PCN
    slot = newslot[dst2]
    tile_of = slot // P
    slot_in = slot % P
    sslot = newslot[src2]
    gpid = _gpid(score, sslot)
    region_of = np.digitize(gpid, GB[1:3]).astype(np.int64)

    NR = 3
    counts = np.zeros((NCORE, NT, NR), np.int64)
    np.add.at(counts, (core_of, tile_of, region_of), 1)
    kmax = counts.max(axis=0)                      # [NT, NR]
    K = ((kmax + P - 1) // P) * P
    K[kmax == 0] = 0

    order = np.lexsort((region_of, tile_of, core_of))
    so_gpid = gpid[order]
    so_norm = normf[order]
    so_slot = slot_in[order]
    so_core = core_of[order]
    so_tile = tile_of[order]
    so_reg = region_of[order]

    icol = np.zeros((NT, NR), np.int64)
    cbase = np.zeros((NT, NR), np.int64)
    ic = cc = 0
    for t in range(NT):
        for r in range(NR):
            icol[t, r] = ic
            cbase[t, r] = cc
            ic += K[t, r] // 16
            cc += K[t, r] // P
    idxcols, nch = ic, cc

    # full padded X in gpid layout (shared by all cores)
    x_np = np.asarray(x, dtype=np.float32)
    xf = np.zeros((NPT, FEAT), np.float16)
    nodes = np.arange(N, dtype=np.int64)
    xf[_gpid(nodes // PCN, newslot)] = x_np.astype(np.float16)

    # weight packs
    w_list = [np.asarray(w, np.float32) for w in Ws] + [np.asarray(Wp, np.float32)]
    b_list = [np.asarray(b, np.float32) for b in bs] + [np.asarray(bp, np.float32)]
    # W1 [512,2048] as lhsT blocks (kb in 4, mb in 16): col (kb*16+mb)*128
    W1 = w_list[0]
    w1p = np.zeros((P, 4 * 16 * P), np.float16)
    for kb in range(4):
        for mb in range(16):
            w1p[:, (kb * 16 + mb) * P:(kb * 16 + mb + 1) * P] = (
                W1[kb * P:(kb + 1) * P, mb * P:(mb + 1) * P].astype(np.float16))
    # W2..W5, Wp as rhs blocks (kb-major): [128, nk*Md]
    def rhs_pack(Wr, Kd, Md):
        wp_ = np.zeros((Kd, Md), np.float32)
        wp_[: Wr.shape[0], : Wr.shape[1]] = Wr
        nk = Kd // P
        out = np.zeros((P, nk * Md), np.float16)
        for kb in range(nk):
            out[:, kb * Md:(kb + 1) * Md] = wp_[kb * P:(kb + 1) * P].astype(np.float16)
        return out

    w2p = rhs_pack(w_list[1], 2048, 1024)
    w3p = rhs_pack(w_list[2], 1024, 512)
    w4p = rhs_pack(w_list[3], 512, 128)
    w5p = rhs_pack(w_list[4], 128, 128)
    wpp = rhs_pack(w_list[5], 128, 4)

    # biases as per-partition columns [128, nblk]
    def bias_cols(b, width):
        bb = np.zeros(width, np.float32)
        bb[: b.shape[0]] = b
        return bb.reshape(width // P, P).T.astype(np.float16).copy()

    b1c = bias_cols(b_list[0], 2048)   # d0 epilogue (relu)
    b2c = bias_cols(b_list[1], 1024)   # agg1 epilogue
    b3c = bias_cols(b_list[2], 512)    # agg2
    b4c = bias_cols(b_list[3], 128)    # agg3
    b5c = bias_cols(b_list[4], 128)    # agg4
    bpr = np.zeros((1, 4), np.float16)
    bpr[0, :3] = b_list[5].astype(np.float16)

    iota = np.tile(np.arange(P, dtype=np.float16), (P, 1))
    ident = np.eye(P, dtype=np.float16)
    ones1 = np.ones((1, P), np.float16)

    core_starts = np.searchsorted(so_core, np.arange(NCORE + 1))
    in_maps = []
    for c in range(NCORE):
        lo, hi = core_starts[c], core_starts[c + 1]
        ct, cr = so_tile[lo:hi], so_reg[lo:hi]
        cg, cn, cs = so_gpid[lo:hi], so_norm[lo:hi], so_slot[lo:hi]
        idx16 = np.zeros((16, idxcols), np.int16)
        slotp = np.zeros((P, nch), np.float16)
        normp = np.zeros((P, nch), np.float16)
        pos = 0
        for t in range(NT):
            for r in range(NR):
                k = int(K[t, r])
                if k == 0:
                    continue
                n_e = int(counts[c, t, r])
                seg = slice(pos, pos + n_e)
                assert np.all(ct[seg] == t) and np.all(cr[seg] == r), (c, t, r)
                reg_lo = GB[r]
                arr = np.zeros(k, np.int64)
                arr[:n_e] = cg[seg] - reg_lo
                assert 0 <= arr.min(initial=0) and arr.max(initial=0) < 32768
                idx16[:, int(icol[t, r]): int(icol[t, r]) + k // 16] = (
                    arr.reshape(k // 16, 16).T.astype(np.int16))
                sl = np.zeros(k, np.float32)
                sl[:n_e] = cs[seg]
                nm = np.zeros(k, np.float32)
                nm[:n_e] = cn[seg]
                cb = int(cbase[t, r])
                slotp[:, cb: cb + k // P] = (
                    sl.reshape(k // P, P).T.astype(np.float16))
                normp[:, cb: cb + k // P] = (
                    nm.reshape(k // P, P).T.astype(np.float16))
                pos += n_e
        assert pos == hi - lo

        # full S (one-hot x norm), for layers that load S instead of
        # building it on DVE
        oh = (slotp[:, :, None] == np.arange(P, dtype=np.float16)[None, None, :])
        sfull = (oh * normp[:, :, None]).astype(np.float16).reshape(P, nch * P)
        m = {
            "xf": xf,
            "idx16": np.tile(idx16, (8, 1)),
            "slotp": slotp,
            "normp": normp,
            "sfull": sfull,
            "iota": iota,
            "ident": ident,
            "ones1": ones1,
            "w1": w1p, "w2": w2p, "w3": w3p, "w4": w4p, "w5": w5p, "wp": wpp,
            "b1": b1c, "b2": b2c, "b3": b3c, "b4": b4c, "b5": b5c, "bp": bpr,
        }
        in_maps.append(m)

    meta = dict(K=K, icol=icol, cbase=cbase, idxcols=idxcols, nch=nch,
                newslot=newslot)
    return in_maps, meta


# ---------------------------------------------------------------- program
def _bc3(ap, ncols, inner=P, mode="col"):
    base = ap.ap
    if mode == "col":
        return bass.AP(ap.tensor, ap.offset, [base[0], [1, ncols], [0, inner]])
    else:
        return bass.AP(ap.tensor, ap.offset, [base[0], [0, ncols], base[1]])


def _3d(ap, ncols, inner=P):
    return bass.AP(ap.tensor, ap.offset, [ap.ap[0], [inner, ncols], [1, inner]])


def build_program(meta):
    import concourse.bacc as bacc
    nc = bacc.Bacc("TRN2", num_swdge_queues=4)
    K, icol, cbase = meta["K"], meta["icol"], meta["cbase"]
    idxcols, nch = meta["idxcols"], meta["nch"]
    rg = [list(range(NCORE))]

    # ---------------- params
    pr = {}
    def par(name, shape, dt):
        pr[name] = nc.declare_dram_parameter(name, shape, dt, isOutput=False)
    par("xf", [NPT, FEAT], FP16)
    par("idx16", [P, idxcols], I16)
    par("slotp", [P, nch], FP16)
    par("normp", [P, nch], FP16)
    par("sfull", [P, nch * P], FP16)
    par("iota", [P, P], FP16)
    par("ident", [P, P], FP16)
    par("ones1", [1, P], FP16)
    par("w1", [P, 4 * 16 * P], FP16)
    par("w2", [P, 16 * 1024], FP16)
    par("w3", [P, 8 * 512], FP16)
    par("w4", [P, 4 * 128], FP16)
    par("w5", [P, 128], FP16)
    par("wp", [P, 4], FP16)
    par("b1", [P, 16], FP16)
    par("b2", [P, 8], FP16)
    par("b3", [P, 4], FP16)
    par("b4", [P, 1], FP16)
    par("b5", [P, 1], FP16)
    par("bp", [1, 4], FP16)
    out_ext = nc.declare_dram_parameter("out", [NPC, 3], F32, isOutput=True)
    import os
    DBG = bool(int(os.environ.get("KDBG", "0")))
    dbg = {}
    if DBG:
        for nm, shape in [("dbg_g1", [NT * P, 512]), ("dbg_h1", [NT * P, 2048]),
                          ("dbg_t2", [NPC, 1024]), ("dbg_h2", [NT * P, 1024]),
                          ("dbg_t3", [NPC, 512]), ("dbg_t4", [NPC, 128]),
                          ("dbg_t5", [NPC, 128])]:
            dbg[nm] = nc.declare_dram_parameter(nm, shape, FP16, isOutput=True)

    # ---------------- internal DRAM
    # layers l=1..4 aggregate T_{l+1}; width WG[l]
    town = {}   # (l, chunk j) -> per-core town tensor
    for l in range(1, 5):
        for j in range(3):
            town[l, j] = nc.dram_tensor(
                f"tn{l}_{j}", [CH[j + 1] - CH[j], WG[l]], FP16)
    TF = {}
    for l in range(1, 5):
        for j in range(3):
            TF[l, j] = nc.dram_tensor(
                f"tf{l}_{j}", [(GB[j + 1] - GB[j]), WG[l]], FP16,
                addr_space="Shared")
    PART = {(l, p): nc.dram_tensor(f"part{l}_{p}", [NT * P, WG[l]], FP16)
            for l in range(1, 5) for p in range(2)}

    with tile.TileContext(nc) as tc:
        import contextlib
        with contextlib.ExitStack() as ctx:
            cpool = ctx.enter_context(tc.tile_pool(name="const", bufs=1))
            msgp = ctx.enter_context(tc.tile_pool(name="msg", bufs=7))
            spool = ctx.enter_context(tc.tile_pool(name="sb", bufs=3))
            pp = ctx.enter_context(tc.tile_pool(name="ps", bufs=2, space="PSUM"))
            hp = ctx.enter_context(tc.tile_pool(name="hh", bufs=2))

            # ---- resident constants
            def cload(name, shape, dt):
                t_ = cpool.tile(shape, dt, tag=name, name=name)
                nc.sync.dma_start(out=t_[:], in_=pr[name][:])
                return t_
            idx_sb = cload("idx16", [P, idxcols], I16)
            slot_sb = cload("slotp", [P, nch], FP16)
            norm_sb = cload("normp", [P, nch], FP16)
            iota_sb = cload("iota", [P, P], FP16)
            ident_sb = cload("ident", [P, P], FP16)
            ones_sb = cload("ones1", [1, P], FP16)
            w1_sb = cload("w1", [P, 4 * 16 * P], FP16)
            w2_sb = cload("w2", [P, 16 * 1024], FP16)
            w3_sb = cload("w3", [P, 8 * 512], FP16)
            w4_sb = cload("w4", [P, 4 * 128], FP16)
            w5_sb = cload("w5", [P, 128], FP16)
            wp_sb = cload("wp", [P, 4], FP16)
            b1_sb = cload("b1", [P, 16], FP16)
            b2_sb = cload("b2", [P, 8], FP16)
            b3_sb = cload("b3", [P, 4], FP16)
            b4_sb = cload("b4", [P, 1], FP16)
            b5_sb = cload("b5", [P, 1], FP16)
            bp_sb = cload("bp", [1, 4], FP16)

            qn = [0]

            def build_s(t, r_list, load=False):
                """One S tile covering the chunks of (tile t, regions r_list)
                (contiguous in cbase layout). load=True DMAs the precomputed
                S from DRAM instead of building it on DVE."""
                c_lo = int(cbase[t, r_list[0]])
                ctn = sum(int(K[t, r]) // P for r in r_list)
                s_t = spool.tile([P, max(ctn, 1) * P], FP16, tag="s", name="s_t")
                if load:
                    if ctn > 0:
                        nc.sync.dma_start(
                            out=s_t[:, : ctn * P],
                            in_=pr["sfull"][:, c_lo * P:(c_lo + ctn) * P])
                    return s_t, ctn
                for b0 in range(0, ctn, SBATCH):
                    nb = min(SBATCH, ctn - b0)
                    cb0 = c_lo + b0
                    o3 = _3d(s_t[:, b0 * P:(b0 + nb) * P], nb)
                    nc.vector.tensor_tensor(
                        out=o3,
                        in0=_bc3(slot_sb[:, cb0:cb0 + nb], nb, mode="col"),
                        in1=_bc3(iota_sb[:], nb, mode="mat"),
                        op=mybir.AluOpType.is_equal)
                    nc.vector.tensor_tensor(
                        out=o3, in0=o3,
                        in1=_bc3(norm_sb[:, cb0:cb0 + nb], nb, mode="col"),
                        op=mybir.AluOpType.mult)
                return s_t, ctn

            def gather_mm(t, r, src_dram, W, s_t, s_coff, ps, mmcnt, mmtot,
                          use_start=True):
                """Gathers + aggregation matmuls for (tile t, region r).
                s_coff: chunk offset of this region within s_t.
                mmcnt: chunks already accumulated into ps; mmtot: total
                expected (stop flags on the last). use_start=False when the
                psum was already initialized (identity re-add). Returns new
                mmcnt."""
                k = int(K[t, r])
                if k == 0:
                    return mmcnt
                nf = W // P
                gmax = GMAX if W <= 512 else (4096 // W)  # cap msg at 8KB/part
                pos = 0
                while pos < k:
                    ks = min(gmax * P, k - pos)
                    ic = int(icol[t, r]) + pos // 16
                    msg = msgp.tile([P, 4096], FP16, tag="msg", name="msg")
                    nc.gpsimd.dma_gather(
                        out_ap=_3d(msg[:, : (ks // P) * W], ks // P, inner=W),
                        in_ap=src_dram,
                        idxs_ap=idx_sb[:, ic: ic + ks // 16],
                        num_idxs=ks,
                        num_idxs_reg=ks,
                        elem_size=W,
                        elem_step=W,
                        queue_num=qn[0],
                    )
                    qn[0] = (qn[0] + 1) % 4
                    for ci in range(ks // P):
                        cglob = s_coff + pos // P + ci
                        for fb in range(nf):
                            # start only on the first write to each 2KB PSUM
                            # zero region (512 f32 cols = 4 fb slices)
                            nc.tensor.matmul(
                                out=ps[:, fb * P:(fb + 1) * P],
                                lhsT=msg[:, ci * W + fb * P: ci * W + (fb + 1) * P],
                                rhs=s_t[:, cglob * P:(cglob + 1) * P],
                                start=(use_start and mmcnt == 0
                                       and fb % 4 == 0),
                                stop=(mmcnt == mmtot - 1))
                        mmcnt += 1
                    pos += ks
                return mmcnt

            def store_town(l, t, t_sb, W):
                j = 0 if t < 16 else (1 if t < 32 else 2)
                r0 = t * P - CH[j]
                nc.sync.dma_start(
                    out=town[l, j][r0:r0 + P, :], in_=t_sb[:, :W])

            def emit_ag(l, j):
                nc.gpsimd.collective_compute(
                    "AllGather", mybir.AluOpType.bypass, replica_groups=rg,
                    ins=[town[l, j][:]], outs=[TF[l, j][:]])

            # ================= phase 1: agg0(X) + d0 + d1 -> T2 =================
            def phase1_tile(t):
                s_t, ctn = build_s(t, [0, 1, 2])
                ps = pp.tile([P, 1024], F32, tag="agg", name="ps_agg")
                mm = 0
                coff = 0
                for r in range(3):
                    mm = gather_mm(t, r, pr["xf"][GB[r]:GB[r + 1], :], 512,
                                   s_t, coff, ps, mm, ctn)
                    coff += int(K[t, r]) // P
                assert mm == ctn and ctn > 0
                g1t = hp.tile([P, 512], FP16, tag="g1t", name="g1t")
                nc.scalar.activation(
                    out=g1t[:], in_=ps[:, :512],
                    func=mybir.ActivationFunctionType.Copy)
                if DBG:
                    nc.sync.dma_start(out=dbg["dbg_g1"][t * P:(t + 1) * P, :],
                                      in_=g1t[:])
                # d0: H1^T = relu(W1^T-blocks @ G1^T + b1), 4 quarters
                h1t = hp.tile([P, 2048], FP16, tag="h1t", name="h1t")
                for q in range(4):
                    ps0 = pp.tile([P, 512], F32, tag="d0", name="ps_d0")
                    for mi in range(4):
                        mb = q * 4 + mi
                        for kb in range(4):
                            nc.tensor.matmul(
                                out=ps0[:, mi * P:(mi + 1) * P],
                                lhsT=w1_sb[:, (kb * 16 + mb) * P:(kb * 16 + mb + 1) * P],
                                rhs=g1t[:, kb * P:(kb + 1) * P],
                                start=(kb == 0 and mi == 0),
                                stop=(kb == 3))
                    for mi in range(4):
                        mb = q * 4 + mi
                        nc.scalar.activation(
                            out=h1t[:, mb * P:(mb + 1) * P],
                            in_=ps0[:, mi * P:(mi + 1) * P],
                            func=mybir.ActivationFunctionType.Relu,
                            bias=b1_sb[:, mb:mb + 1])
                # d1: T2 = H1 @ W2 (normal orientation), 2 halves of 512
                t2sb = hp.tile([P, 1024], FP16, tag="tout", name="t2sb")
                for h in range(2):
                    psd = pp.tile([P, 512], F32, tag="d", name="ps_d")
                    for kb in range(16):
                        nc.tensor.matmul(
                            out=psd[:],
                            lhsT=h1t[:, kb * P:(kb + 1) * P],
                            rhs=w2_sb[:, kb * 1024 + h * 512: kb * 1024 + h * 512 + 512],
                            start=(kb == 0), stop=(kb == 15))
                    nc.scalar.activation(
                        out=t2sb[:, h * 512:(h + 1) * 512], in_=psd[:],
                        func=mybir.ActivationFunctionType.Copy)
                if DBG:
                    nc.sync.dma_start(out=dbg["dbg_h1"][t * P:(t + 1) * P, :],
                                      in_=h1t[:])
                    nc.sync.dma_start(out=dbg["dbg_t2"][t * P:(t + 1) * P, :],
                                      in_=t2sb[:])
                store_town(1, t, t2sb, 1024)
                if t == 15:
                    emit_ag(1, 0)
                elif t == 31:
                    emit_ag(1, 1)

            # ========= layers l=1..4: agg_l (3 passes, 1 per region) + dense ====
            # agg_l consumes TF[l,*] (width WG[l]), produces H^{l+1,T}; dense
            # d_{l+1} produces T_{l+2} (towns l+1) or the final output. Passes
            # 0/1 stage the PSUM to DRAM as fp16 partials; passes 1/2 re-add
            # them via an identity matmul.
            def agg_tile(l, p, t):
                W = WG[l]
                nf = W // P
                bias_sb = {1: b2_sb, 2: b3_sb, 3: b4_sb, 4: b5_sb}[l]
                last = p == 2
                if True:
                    if True:
                        s_t, ctn = build_s(t, [p],
                                           load=(l >= 3 and t % 2 == 1))
                        ps = pp.tile([P, 1024], F32, tag="agg",
                                     name="ps_agg")
                        if p > 0:
                            pb = hp.tile([P, 1024], FP16, tag="pb", name="pb", bufs=2)
                            nc.sync.dma_start(
                                out=pb[:, :W],
                                in_=PART[l, p - 1][t * P:(t + 1) * P, :])
                            nid = (W + 511) // 512
                            for j in range(nid):
                                w_ = min(512, W - j * 512)
                                nc.tensor.matmul(
                                    out=ps[:, j * 512: j * 512 + w_],
                                    lhsT=ident_sb[:],
                                    rhs=pb[:, j * 512: j * 512 + w_],
                                    start=True,
                                    stop=(ctn == 0 and j == nid - 1))
                            if ctn > 0:
                                gather_mm(t, p, TF[l, p][:], W, s_t, 0, ps,
                                          0, ctn, use_start=False)
                        else:
                            gather_mm(t, 0, TF[l, 0][:], W, s_t, 0, ps, 0,
                                      max(ctn, 1))
                            if ctn == 0:
                                nc.vector.memset(ps[:, :W], 0.0)
                        if not last:
                            pa = hp.tile([P, 1024], FP16, tag="pa", name="pa", bufs=2)
                            nc.scalar.activation(
                                out=pa[:, :W], in_=ps[:, :W],
                                func=mybir.ActivationFunctionType.Copy)
                            nc.sync.dma_start(
                                out=PART[l, p][t * P:(t + 1) * P, :],
                                in_=pa[:, :W])
                            return
                        hT = hp.tile([P, 1024], FP16, tag="ht", name="hT", bufs=3)
                        for fb in range(nf):
                            nc.scalar.activation(
                                out=hT[:, fb * P:(fb + 1) * P],
                                in_=ps[:, fb * P:(fb + 1) * P],
                                func=mybir.ActivationFunctionType.Relu,
                                bias=bias_sb[:, fb:fb + 1])
                        if DBG and l == 1:
                            nc.sync.dma_start(
                                out=dbg["dbg_h2"][t * P:(t + 1) * P, :],
                                in_=hT[:, :1024])
                        # dense d_{l+1}
                        if l == 1:
                            # H2[1024] @ W3 -> T3 [512]
                            t3 = hp.tile([P, 512], FP16, tag="tout", name="t3")
                            psd = pp.tile([P, 512], F32, tag="d", name="ps_d")
                            for kb in range(8):
                                nc.tensor.matmul(
                                    out=psd[:],
                                    lhsT=hT[:, kb * P:(kb + 1) * P],
                                    rhs=w3_sb[:, kb * 512:(kb + 1) * 512],
                                    start=(kb == 0), stop=(kb == 7))
                            nc.scalar.activation(
                                out=t3[:], in_=psd[:],
                                func=mybir.ActivationFunctionType.Copy)
                            if DBG:
                                nc.sync.dma_start(
                                    out=dbg["dbg_t3"][t * P:(t + 1) * P, :],
                                    in_=t3[:])
                            store_town(2, t, t3, 512)
                            if t == 15:
                                emit_ag(2, 0)
                            elif t == 31:
                                emit_ag(2, 1)
                        elif l == 2:
                            # H3[512] @ W4 -> T4 [128]
                            t4 = hp.tile([P, 128], FP16, tag="tout4", name="t4")
                            psd = pp.tile([P, 512], F32, tag="d", name="ps_d")
                            for kb in range(4):
                                nc.tensor.matmul(
                                    out=psd[:, :128],
                                    lhsT=hT[:, kb * P:(kb + 1) * P],
                                    rhs=w4_sb[:, kb * 128:(kb + 1) * 128],
                                    start=(kb == 0), stop=(kb == 3))
                            nc.scalar.activation(
                                out=t4[:], in_=psd[:, :128],
                                func=mybir.ActivationFunctionType.Copy)
                            if DBG:
                                nc.sync.dma_start(
                                    out=dbg["dbg_t4"][t * P:(t + 1) * P, :],
                                    in_=t4[:])
                            store_town(3, t, t4, 128)
                            if t == 15:
                                emit_ag(3, 0)
                            elif t == 31:
                                emit_ag(3, 1)
                        elif l == 3:
                            # H4[128] @ W5 -> T5 [128]
                            t5 = hp.tile([P, 128], FP16, tag="tout4", name="t5")
                            psd = pp.tile([P, 512], F32, tag="d", name="ps_d")
                            nc.tensor.matmul(
                                out=psd[:, :128], lhsT=hT[:, :128],
                                rhs=w5_sb[:], start=True, stop=True)
                            nc.scalar.activation(
                                out=t5[:], in_=psd[:, :128],
                                func=mybir.ActivationFunctionType.Copy)
                            if DBG:
                                nc.sync.dma_start(
                                    out=dbg["dbg_t5"][t * P:(t + 1) * P, :],
                                    in_=t5[:])
                            store_town(4, t, t5, 128)
                            if t == 15:
                                emit_ag(4, 0)
                            elif t == 31:
                                emit_ag(4, 1)
                        else:
                            # d5: out = H5 @ Wp + bp
                            psd = pp.tile([P, 512], F32, tag="d", name="ps_d")
                            nc.tensor.matmul(
                                out=psd[:, :4], lhsT=hT[:, :128], rhs=wp_sb[:],
                                start=True, stop=False)
                            nc.tensor.matmul(
                                out=psd[:, :4], lhsT=ones_sb[0:1, :],
                                rhs=bp_sb[0:1, :], start=False, stop=True)
                            osb = hp.tile([P, 4], F32, tag="fout", name="osb")
                            nc.vector.tensor_copy(out=osb[:], in_=psd[:, :4])
                            nc.sync.dma_start(
                                out=out_ext[t * P:(t + 1) * P, :],
                                in_=osb[:, :3])

            # ---------------- emission driver (software pipelining) ----------
            # phase 1 tiles 0..33, then interleave its tail with agg1-pass0
            # (whose gathers wait on the first T2 AllGather chunk).
            for t in range(34):
                phase1_tile(t)
            j = 0
            for t in range(34, NT):
                phase1_tile(t)
                while j < 2 * (t - 33) and j < NT:
                    agg_tile(1, 0, j)
                    j += 1
            emit_ag(1, 2)
            while j < NT:
                agg_tile(1, 0, j)
                j += 1
            for p in (1, 2):
                for t in range(NT):
                    agg_tile(1, p, t)
            emit_ag(2, 2)
            for l in range(2, 5):
                for p in range(3):
                    for t in range(NT):
                        agg_tile(l, p, t)
                if l < 4:
                    emit_ag(l + 1, 2)

    nc.finalize()
    return nc


# ------------------------------------------------------------------ driver
_CACHE = {}


def kernel(x, edge_index, edge_attr, W1, b1, W2, b2, W3, b3, W4, b4, W5, b5,
           Wp, bp):
    apply_tile_patch()
    import os
    from concourse.bass_utils import run_bass_kernel_spmd

    Ws = [W1, W2, W3, W4, W5]
    bs = [b1, b2, b3, b4, b5]
    in_maps, meta = preprocess(x, edge_index, edge_attr, Ws, bs, Wp, bp)

    key = (meta["K"].tobytes(), meta["nch"], meta["idxcols"])
    nc = _CACHE.get(key)
    if nc is None:
        nc = build_program(meta)
        _CACHE[key] = nc

    res = run_bass_kernel_spmd(
        nc, in_maps, core_ids=list(range(NCORE)),
        trace=bool(int(os.environ.get("TRACE", "0"))))
    if res.exec_time_ns:
        print(f"HW exec time: {res.exec_time_ns} ns")
    newslot = meta["newslot"]
    N = NCORE * PCN
    out = np.empty((N, 3), np.float32)
    for c in range(NCORE):
        rows = res.results[c]["out"]
        sel = newslot[c * PCN:(c + 1) * PCN]
        out[c * PCN:(c + 1) * PCN] = rows[sel]
    return np.ascontiguousarray(out)


# revision 46
# speedup vs baseline: 1.0075x; 1.0075x over previous
"""GCN (5-layer ColorGNN) Bass kernel for 8 TRN2 NeuronCores — v2.

Pipelined design (node-sharded SPMD, 6272 padded nodes/core = 49 tiles):

  - Full padded X (fp16, gpid layout) is staged on EVERY core: the layer-1
    aggregation gathers it directly -> no AllGather for layer 1.
  - Aggregation outputs are produced TRANSPOSED (H^T tiles: features on
    partitions, 128 dst nodes on free): out^T[f,n] = sum_e msg[e,f]*S[e,n]
    with lhsT=msg (gathered rows), rhs=S (one-hot x norm). The full
    symmetric norm dinv[src]*ew*dinv[dst] is folded into S on the host, and
    the bias is a per-partition ACT bias in the relu epilogue.
  - H^T tiles feed the next dense matmul as lhsT straight from SBUF: no
    transposes, no H round-trips through DRAM. Dense outputs T (node-major)
    go to DRAM only as AllGather input. Layer-1's dense pair runs in the
    transposed orientation (lhsT = W1 blocks) so the chain stays in H^T.
  - Each T AllGather is split in 2 node-chunks (4096+2176 rows/core,
    matching gpid regions [0,32768) and [32768,50176) which also serve as
    the int16 gather-index regions). Chunk0 fires when dense tiles 0..31
    are stored; the next layer's aggregation runs in 2 passes (region A
    then region B) so its region-A gathers need only chunk0 -> chunk1's
    transfer hides under the region-A gather. Pass A's PSUM is staged to
    DRAM as an fp16 partial and re-added in pass B via an identity matmul.
  - Host preprocessing: edges bucketed per (core=dst core, dst tile, src
    region), padded to 128-multiples of the cross-core max so all 8 cores
    run one SPMD program.
"""

import numpy as np
import concourse.bass as bass
import concourse.mybir as mybir
import concourse.tile as tile

FP16 = mybir.dt.float16
F32 = mybir.dt.float32
I16 = mybir.dt.int16

P = 128
NCORE = 8
PCN = 6250            # real nodes per core
NPC = 6272            # padded nodes per core (49 tiles)
NT = 49
# AllGather chunks per core (tile-aligned): tiles 0-15 / 16-31 / 32-48
CH = [0, 2048, 4096, 6272]       # per-core row boundaries
GB = [0, 16384, 32768, 50176]    # global gpid base of each chunk
REG0 = 32768          # gpid boundary of gather region 0 (chunks 0a+0b)
NPT = NCORE * NPC     # 50176 padded total nodes
T_SPLIT = 32          # first tile of gather-region 1
GMAX = 8              # chunks per gather call (1024 idx ucode cap)
SBATCH = 8            # chunks per batched S-build DVE op

FEAT = 512
WG = [512, 1024, 512, 128, 128]   # gather/agg width per layer l = 0..4
# dense widths: d0: 512->2048(T out), d1: 2048->1024, d2: 1024->512,
#               d3: 512->128, d4: 128->128, d5: 128->4


# ---------------------------------------------------------------- tile patch
def apply_tile_patch():
    """This walrus build allows only 1 sync-wait per Drain; split the tail
    drain's waits across a chain of drains."""
    import bass_rust

    def _drain_and_barrier_split(self, tick_clock, wait_clock):
        from bass_rust import ScopedClock
        drain_inst = self.nc.sync.drain()
        wait_clock.add_sem_waits(
            drain_inst.ins, ScopedClock({None: tick_clock.global_clock})
        )
        si = drain_inst.ins.sync_info
        waits = list(si.on_wait) if si is not None else []
        if len(waits) > 1:
            si.on_wait = [waits[0]]
            for w in waits[1:]:
                extra = self.nc.sync.drain()
                if extra.ins.sync_info is None:
                    extra.ins.sync_info = bass_rust.SyncInfo(
                        on_wait=[w], on_update=[])
                else:
                    extra.ins.sync_info.on_wait = [w]
        self.nc.all_engine_barrier()
        popped = self.nc._tile_sem_poison_stack.pop()
        assert popped is self._sem_poison
        self.nc.clear_and_free_semaphores(list(self.sems.allocated().values()))
        self.nc.all_engine_barrier()

    tile.TileContext._drain_and_barrier = _drain_and_barrier_split


def _gpid(core, slot):
    """Chunk-major padded global id (vectorized over 3 AG chunks)."""
    out = np.empty_like(np.broadcast_arrays(core, slot)[1])
    core = np.asarray(core)
    slot = np.asarray(slot)
    for j in range(3):
        m = (slot >= CH[j]) & (slot < CH[j + 1])
        w = CH[j + 1] - CH[j]
        out[m] = GB[j] + core[m] * w + (slot[m] - CH[j])
    return out


# ------------------------------------------------------------- preprocess
def preprocess(x, edge_index, edge_attr, Ws, bs, Wp, bp):
    """Host-side: normalization, edge sharding/sorting/packing, weight packs.
    Returns (in_maps, meta)."""
    N = NCORE * PCN
    src = np.asarray(edge_index[0], dtype=np.int64)
    dst = np.asarray(edge_index[1], dtype=np.int64)
    ew = np.asarray(edge_attr, dtype=np.float32)
    loop = np.arange(N, dtype=np.int64)
    src2 = np.concatenate([src, loop])
    dst2 = np.concatenate([dst, loop])
    ew2 = np.concatenate([ew, np.ones(N, np.float32)])

    deg = np.bincount(dst2, weights=ew2.astype(np.float64), minlength=N)
    deg = deg.astype(np.float32)
    dinv = np.where(deg > 0, 1.0 / np.sqrt(deg), 0.0).astype(np.float32)
    normf = (dinv[src2] * ew2 * dinv[dst2]).astype(np.float32)  # full norm

    score = src2 // PCN

    # ---- degree-balanced node->tile packing (within each (core, chunk)):
    # re-bin nodes into tiles so per-(tile, region) edge counts are nearly
    # equal across cores and land just under multiples of 128, minimizing
    # SPMD padding. A node keeps its (core, chunk), so src regions are
    # unchanged by the permutation.
    allslot = np.arange(N, dtype=np.int64) % PCN
    chunk_of_node = np.digitize(allslot, CH[1:3])          # [N] in {0,1,2}
    src_chunk = chunk_of_node[src2]
    dvec = np.zeros((N, 3), np.int64)
    np.add.at(dvec, (dst2, src_chunk), 1)

    # per-(core, chunk, r) totals -> global per-(tile-in-chunk, r) caps
    core_all = np.arange(N, dtype=np.int64) // PCN
    Tcr = np.zeros((NCORE, 3, 3), np.int64)
    for r in range(3):
        np.add.at(Tcr, (core_all, chunk_of_node, r), dvec[:, r])
    newslot = np.zeros(N, np.int64)
    for j in range(3):
        nt_j = (CH[j + 1] - CH[j]) // P
        maxT = Tcr[:, j, :].max(axis=0)                    # [3]
        caps = np.zeros((nt_j, 3), np.int64)
        for r in range(3):
            m_tot = int(np.ceil((maxT[r] + 16 * nt_j) / P))
            base, extra = divmod(m_tot, nt_j)
            caps[:, r] = P * base
            caps[:extra, r] += P
        for c in range(NCORE):
            vs = np.where((core_all == c) & (chunk_of_node == j))[0]
            dv = dvec[vs]
            order_v = np.argsort(-dv.sum(axis=1), kind="stable")
            used = np.zeros((nt_j, 3), np.float64)
            cnt = np.zeros(nt_j, np.int64)
            pos = np.zeros(nt_j, np.int64)
            for vi in order_v:
                nd = dv[vi]
                ok = cnt < P
                over = np.maximum(used + nd - caps, 0).sum(axis=1)
                frac = ((used + nd) / caps).max(axis=1)
                tscore = over * 1e9 + frac
                tscore[~ok] = np.inf
                ti = int(tscore.argmin())
                used[ti] += nd
                cnt[ti] += 1
                newslot[vs[vi]] = CH[j] + ti * P + pos[ti]
                pos[ti] += 1

    core_of = dst2 // PCN
    slot = newslot[dst2]
    tile_of = slot // P
    slot_in = slot % P
    sslot = newslot[src2]
    gpid = _gpid(score, sslot)
    region_of = np.digitize(gpid, GB[1:3]).astype(np.int64)

    NR = 3
    counts = np.zeros((NCORE, NT, NR), np.int64)
    np.add.at(counts, (core_of, tile_of, region_of), 1)
    kmax = counts.max(axis=0)                      # [NT, NR]
    K = ((kmax + P - 1) // P) * P
    K[kmax == 0] = 0

    order = np.lexsort((region_of, tile_of, core_of))
    so_gpid = gpid[order]
    so_norm = normf[order]
    so_slot = slot_in[order]
    so_core = core_of[order]
    so_tile = tile_of[order]
    so_reg = region_of[order]

    icol = np.zeros((NT, NR), np.int64)
    cbase = np.zeros((NT, NR), np.int64)
    ic = cc = 0
    for t in range(NT):
        for r in range(NR):
            icol[t, r] = ic
            cbase[t, r] = cc
            ic += K[t, r] // 16
            cc += K[t, r] // P
    idxcols, nch = ic, cc

    # full padded X in gpid layout (shared by all cores)
    x_np = np.asarray(x, dtype=np.float32)
    xf = np.zeros((NPT, FEAT), np.float16)
    nodes = np.arange(N, dtype=np.int64)
    xf[_gpid(nodes // PCN, newslot)] = x_np.astype(np.float16)

    # weight packs
    w_list = [np.asarray(w, np.float32) for w in Ws] + [np.asarray(Wp, np.float32)]
    b_list = [np.asarray(b, np.float32) for b in bs] + [np.asarray(bp, np.float32)]
    # W1 [512,2048] as lhsT blocks (kb in 4, mb in 16): col (kb*16+mb)*128
    W1 = w_list[0]
    w1p = np.zeros((P, 4 * 16 * P), np.float16)
    for kb in range(4):
        for mb in range(16):
            w1p[:, (kb * 16 + mb) * P:(kb * 16 + mb + 1) * P] = (
                W1[kb * P:(kb + 1) * P, mb * P:(mb + 1) * P].astype(np.float16))
    # W2..W5, Wp as rhs blocks (kb-major): [128, nk*Md]
    def rhs_pack(Wr, Kd, Md):
        wp_ = np.zeros((Kd, Md), np.float32)
        wp_[: Wr.shape[0], : Wr.shape[1]] = Wr
        nk = Kd // P
        out = np.zeros((P, nk * Md), np.float16)
        for kb in range(nk):
            out[:, kb * Md:(kb + 1) * Md] = wp_[kb * P:(kb + 1) * P].astype(np.float16)
        return out

    w2p = rhs_pack(w_list[1], 2048, 1024)
    w3p = rhs_pack(w_list[2], 1024, 512)
    w4p = rhs_pack(w_list[3], 512, 128)
    w5p = rhs_pack(w_list[4], 128, 128)
    wpp = rhs_pack(w_list[5], 128, 4)

    # biases as per-partition columns [128, nblk]
    def bias_cols(b, width):
        bb = np.zeros(width, np.float32)
        bb[: b.shape[0]] = b
        return bb.reshape(width // P, P).T.astype(np.float16).copy()

    b1c = bias_cols(b_list[0], 2048)   # d0 epilogue (relu)
    b2c = bias_cols(b_list[1], 1024)   # agg1 epilogue
    b3c = bias_cols(b_list[2], 512)    # agg2
    b4c = bias_cols(b_list[3], 128)    # agg3
    b5c = bias_cols(b_list[4], 128)    # agg4
    bpr = np.zeros((1, 4), np.float16)
    bpr[0, :3] = b_list[5].astype(np.float16)

    iota = np.tile(np.arange(P, dtype=np.float16), (P, 1))
    ident = np.eye(P, dtype=np.float16)
    ones1 = np.ones((1, P), np.float16)

    core_starts = np.searchsorted(so_core, np.arange(NCORE + 1))
    in_maps = []
    for c in range(NCORE):
        lo, hi = core_starts[c], core_starts[c + 1]
        ct, cr = so_tile[lo:hi], so_reg[lo:hi]
        cg, cn, cs = so_gpid[lo:hi], so_norm[lo:hi], so_slot[lo:hi]
        idx16 = np.zeros((16, idxcols), np.int16)
        slotp = np.zeros((P, nch), np.float16)
        normp = np.zeros((P, nch), np.float16)
        pos = 0
        for t in range(NT):
            for r in range(NR):
                k = int(K[t, r])
                if k == 0:
                    continue
                n_e = int(counts[c, t, r])
                seg = slice(pos, pos + n_e)
                assert np.all(ct[seg] == t) and np.all(cr[seg] == r), (c, t, r)
                reg_lo = GB[r]
                arr = np.zeros(k, np.int64)
                arr[:n_e] = cg[seg] - reg_lo
                assert 0 <= arr.min(initial=0) and arr.max(initial=0) < 32768
                idx16[:, int(icol[t, r]): int(icol[t, r]) + k // 16] = (
                    arr.reshape(k // 16, 16).T.astype(np.int16))
                sl = np.zeros(k, np.float32)
                sl[:n_e] = cs[seg]
                nm = np.zeros(k, np.float32)
                nm[:n_e] = cn[seg]
                cb = int(cbase[t, r])
                slotp[:, cb: cb + k // P] = (
                    sl.reshape(k // P, P).T.astype(np.float16))
                normp[:, cb: cb + k // P] = (
                    nm.reshape(k // P, P).T.astype(np.float16))
                pos += n_e
        assert pos == hi - lo

        # full S (one-hot x norm), for layers that load S instead of
        # building it on DVE
        oh = (slotp[:, :, None] == np.arange(P, dtype=np.float16)[None, None, :])
        sfull = (oh * normp[:, :, None]).astype(np.float16).reshape(P, nch * P)
        m = {
            "xf": xf,
            "idx16": np.tile(idx16, (8, 1)),
            "slotp": slotp,
            "normp": normp,
            "sfull": sfull,
            "iota": iota,
            "ident": ident,
            "ones1": ones1,
            "w1": w1p, "w2": w2p, "w3": w3p, "w4": w4p, "w5": w5p, "wp": wpp,
            "b1": b1c, "b2": b2c, "b3": b3c, "b4": b4c, "b5": b5c, "bp": bpr,
        }
        in_maps.append(m)

    meta = dict(K=K, icol=icol, cbase=cbase, idxcols=idxcols, nch=nch,
                newslot=newslot)
    return in_maps, meta


# ---------------------------------------------------------------- program
def _bc3(ap, ncols, inner=P, mode="col"):
    base = ap.ap
    if mode == "col":
        return bass.AP(ap.tensor, ap.offset, [base[0], [1, ncols], [0, inner]])
    else:
        return bass.AP(ap.tensor, ap.offset, [base[0], [0, ncols], base[1]])


def _3d(ap, ncols, inner=P):
    return bass.AP(ap.tensor, ap.offset, [ap.ap[0], [inner, ncols], [1, inner]])


def build_program(meta):
    import concourse.bacc as bacc
    nc = bacc.Bacc("TRN2", num_swdge_queues=4)
    K, icol, cbase = meta["K"], meta["icol"], meta["cbase"]
    idxcols, nch = meta["idxcols"], meta["nch"]
    rg = [list(range(NCORE))]

    # ---------------- params
    pr = {}
    def par(name, shape, dt):
        pr[name] = nc.declare_dram_parameter(name, shape, dt, isOutput=False)
    par("xf", [NPT, FEAT], FP16)
    par("idx16", [P, idxcols], I16)
    par("slotp", [P, nch], FP16)
    par("normp", [P, nch], FP16)
    par("sfull", [P, nch * P], FP16)
    par("iota", [P, P], FP16)
    par("ident", [P, P], FP16)
    par("ones1", [1, P], FP16)
    par("w1", [P, 4 * 16 * P], FP16)
    par("w2", [P, 16 * 1024], FP16)
    par("w3", [P, 8 * 512], FP16)
    par("w4", [P, 4 * 128], FP16)
    par("w5", [P, 128], FP16)
    par("wp", [P, 4], FP16)
    par("b1", [P, 16], FP16)
    par("b2", [P, 8], FP16)
    par("b3", [P, 4], FP16)
    par("b4", [P, 1], FP16)
    par("b5", [P, 1], FP16)
    par("bp", [1, 4], FP16)
    out_ext = nc.declare_dram_parameter("out", [NPC, 3], F32, isOutput=True)
    import os
    DBG = bool(int(os.environ.get("KDBG", "0")))
    dbg = {}
    if DBG:
        for nm, shape in [("dbg_g1", [NT * P, 512]), ("dbg_h1", [NT * P, 2048]),
                          ("dbg_t2", [NPC, 1024]), ("dbg_h2", [NT * P, 1024]),
                          ("dbg_t3", [NPC, 512]), ("dbg_t4", [NPC, 128]),
                          ("dbg_t5", [NPC, 128])]:
            dbg[nm] = nc.declare_dram_parameter(nm, shape, FP16, isOutput=True)

    # ---------------- internal DRAM
    # layers l=1..4 aggregate T_{l+1}; width WG[l]
    town = {}   # (l, chunk j) -> per-core town tensor
    for l in range(1, 5):
        for j in range(3):
            town[l, j] = nc.dram_tensor(
                f"tn{l}_{j}", [CH[j + 1] - CH[j], WG[l]], FP16)
    TF = {}
    for l in range(1, 5):
        for j in range(3):
            TF[l, j] = nc.dram_tensor(
                f"tf{l}_{j}", [(GB[j + 1] - GB[j]), WG[l]], FP16,
                addr_space="Shared")
    PART = {(l, p): nc.dram_tensor(f"part{l}_{p}", [NT * P, WG[l]], FP16)
            for l in range(1, 5) for p in range(2)}

    with tile.TileContext(nc) as tc:
        import contextlib
        with contextlib.ExitStack() as ctx:
            cpool = ctx.enter_context(tc.tile_pool(name="const", bufs=1))
            msgp = ctx.enter_context(tc.tile_pool(name="msg", bufs=7))
            spool = ctx.enter_context(tc.tile_pool(name="sb", bufs=3))
            pp = ctx.enter_context(tc.tile_pool(name="ps", bufs=2, space="PSUM"))
            hp = ctx.enter_context(tc.tile_pool(name="hh", bufs=2))

            # ---- resident constants
            def cload(name, shape, dt):
                t_ = cpool.tile(shape, dt, tag=name, name=name)
                nc.sync.dma_start(out=t_[:], in_=pr[name][:])
                return t_
            idx_sb = cload("idx16", [P, idxcols], I16)
            slot_sb = cload("slotp", [P, nch], FP16)
            norm_sb = cload("normp", [P, nch], FP16)
            iota_sb = cload("iota", [P, P], FP16)
            ident_sb = cload("ident", [P, P], FP16)
            ones_sb = cload("ones1", [1, P], FP16)
            w1_sb = cload("w1", [P, 4 * 16 * P], FP16)
            w2_sb = cload("w2", [P, 16 * 1024], FP16)
            w3_sb = cload("w3", [P, 8 * 512], FP16)
            w4_sb = cload("w4", [P, 4 * 128], FP16)
            w5_sb = cload("w5", [P, 128], FP16)
            wp_sb = cload("wp", [P, 4], FP16)
            b1_sb = cload("b1", [P, 16], FP16)
            b2_sb = cload("b2", [P, 8], FP16)
            b3_sb = cload("b3", [P, 4], FP16)
            b4_sb = cload("b4", [P, 1], FP16)
            b5_sb = cload("b5", [P, 1], FP16)
            bp_sb = cload("bp", [1, 4], FP16)

            qn = [0]

            def build_s(t, r_list, load=False):
                """One S tile covering the chunks of (tile t, regions r_list)
                (contiguous in cbase layout). load=True DMAs the precomputed
                S from DRAM instead of building it on DVE."""
                c_lo = int(cbase[t, r_list[0]])
                ctn = sum(int(K[t, r]) // P for r in r_list)
                s_t = spool.tile([P, max(ctn, 1) * P], FP16, tag="s", name="s_t")
                if load:
                    if ctn > 0:
                        nc.sync.dma_start(
                            out=s_t[:, : ctn * P],
                            in_=pr["sfull"][:, c_lo * P:(c_lo + ctn) * P])
                    return s_t, ctn
                for b0 in range(0, ctn, SBATCH):
                    nb = min(SBATCH, ctn - b0)
                    cb0 = c_lo + b0
                    o3 = _3d(s_t[:, b0 * P:(b0 + nb) * P], nb)
                    nc.vector.tensor_tensor(
                        out=o3,
                        in0=_bc3(slot_sb[:, cb0:cb0 + nb], nb, mode="col"),
                        in1=_bc3(iota_sb[:], nb, mode="mat"),
                        op=mybir.AluOpType.is_equal)
                    nc.vector.tensor_tensor(
                        out=o3, in0=o3,
                        in1=_bc3(norm_sb[:, cb0:cb0 + nb], nb, mode="col"),
                        op=mybir.AluOpType.mult)
                return s_t, ctn

            def gather_mm(t, r, src_dram, W, s_t, s_coff, ps, mmcnt, mmtot,
                          use_start=True):
                """Gathers + aggregation matmuls for (tile t, region r).
                s_coff: chunk offset of this region within s_t.
                mmcnt: chunks already accumulated into ps; mmtot: total
                expected (stop flags on the last). use_start=False when the
                psum was already initialized (identity re-add). Returns new
                mmcnt."""
                k = int(K[t, r])
                if k == 0:
                    return mmcnt
                nf = W // P
                gmax = GMAX if W <= 512 else (4096 // W)  # cap msg at 8KB/part
                pos = 0
                while pos < k:
                    ks = min(gmax * P, k - pos)
                    ic = int(icol[t, r]) + pos // 16
                    msg = msgp.tile([P, 4096], FP16, tag="msg", name="msg")
                    nc.gpsimd.dma_gather(
                        out_ap=_3d(msg[:, : (ks // P) * W], ks // P, inner=W),
                        in_ap=src_dram,
                        idxs_ap=idx_sb[:, ic: ic + ks // 16],
                        num_idxs=ks,
                        num_idxs_reg=ks,
                        elem_size=W,
                        elem_step=W,
                        queue_num=qn[0],
                    )
                    qn[0] = (qn[0] + 1) % 4
                    for ci in range(ks // P):
                        cglob = s_coff + pos // P + ci
                        for fb in range(nf):
                            # start only on the first write to each 2KB PSUM
                            # zero region (512 f32 cols = 4 fb slices)
                            nc.tensor.matmul(
                                out=ps[:, fb * P:(fb + 1) * P],
                                lhsT=msg[:, ci * W + fb * P: ci * W + (fb + 1) * P],
                                rhs=s_t[:, cglob * P:(cglob + 1) * P],
                                start=(use_start and mmcnt == 0
                                       and fb % 4 == 0),
                                stop=(mmcnt == mmtot - 1))
                        mmcnt += 1
                    pos += ks
                return mmcnt

            def store_town(l, t, t_sb, W):
                j = 0 if t < 16 else (1 if t < 32 else 2)
                r0 = t * P - CH[j]
                nc.sync.dma_start(
                    out=town[l, j][r0:r0 + P, :], in_=t_sb[:, :W])

            def emit_ag(l, j):
                nc.gpsimd.collective_compute(
                    "AllGather", mybir.AluOpType.bypass, replica_groups=rg,
                    ins=[town[l, j][:]], outs=[TF[l, j][:]])

            # ================= phase 1: agg0(X) + d0 + d1 -> T2 =================
            def phase1_tile(t):
                s_t, ctn = build_s(t, [0, 1, 2])
                ps = pp.tile([P, 1024], F32, tag="agg", name="ps_agg")
                mm = 0
                coff = 0
                for r in range(3):
                    mm = gather_mm(t, r, pr["xf"][GB[r]:GB[r + 1], :], 512,
                                   s_t, coff, ps, mm, ctn)
                    coff += int(K[t, r]) // P
                assert mm == ctn and ctn > 0
                g1t = hp.tile([P, 512], FP16, tag="g1t", name="g1t")
                nc.scalar.activation(
                    out=g1t[:], in_=ps[:, :512],
                    func=mybir.ActivationFunctionType.Copy)
                if DBG:
                    nc.sync.dma_start(out=dbg["dbg_g1"][t * P:(t + 1) * P, :],
                                      in_=g1t[:])
                # d0: H1^T = relu(W1^T-blocks @ G1^T + b1), 4 quarters
                h1t = hp.tile([P, 2048], FP16, tag="h1t", name="h1t")
                for q in range(4):
                    ps0 = pp.tile([P, 512], F32, tag="d0", name="ps_d0")
                    for mi in range(4):
                        mb = q * 4 + mi
                        for kb in range(4):
                            nc.tensor.matmul(
                                out=ps0[:, mi * P:(mi + 1) * P],
                                lhsT=w1_sb[:, (kb * 16 + mb) * P:(kb * 16 + mb + 1) * P],
                                rhs=g1t[:, kb * P:(kb + 1) * P],
                                start=(kb == 0 and mi == 0),
                                stop=(kb == 3))
                    for mi in range(4):
                        mb = q * 4 + mi
                        nc.scalar.activation(
                            out=h1t[:, mb * P:(mb + 1) * P],
                            in_=ps0[:, mi * P:(mi + 1) * P],
                            func=mybir.ActivationFunctionType.Relu,
                            bias=b1_sb[:, mb:mb + 1])
                # d1: T2 = H1 @ W2 (normal orientation), 2 halves of 512
                t2sb = hp.tile([P, 1024], FP16, tag="tout", name="t2sb")
                for h in range(2):
                    psd = pp.tile([P, 512], F32, tag="d", name="ps_d")
                    for kb in range(16):
                        nc.tensor.matmul(
                            out=psd[:],
                            lhsT=h1t[:, kb * P:(kb + 1) * P],
                            rhs=w2_sb[:, kb * 1024 + h * 512: kb * 1024 + h * 512 + 512],
                            start=(kb == 0), stop=(kb == 15))
                    nc.scalar.activation(
                        out=t2sb[:, h * 512:(h + 1) * 512], in_=psd[:],
                        func=mybir.ActivationFunctionType.Copy)
                if DBG:
                    nc.sync.dma_start(out=dbg["dbg_h1"][t * P:(t + 1) * P, :],
                                      in_=h1t[:])
                    nc.sync.dma_start(out=dbg["dbg_t2"][t * P:(t + 1) * P, :],
                                      in_=t2sb[:])
                store_town(1, t, t2sb, 1024)
                if t == 15:
                    emit_ag(1, 0)
                elif t == 31:
                    emit_ag(1, 1)

            # ========= layers l=1..4: agg_l (3 passes, 1 per region) + dense ====
            # agg_l consumes TF[l,*] (width WG[l]), produces H^{l+1,T}; dense
            # d_{l+1} produces T_{l+2} (towns l+1) or the final output. Passes
            # 0/1 stage the PSUM to DRAM as fp16 partials; passes 1/2 re-add
            # them via an identity matmul.
            def agg_tile(l, p, t):
                W = WG[l]
                nf = W // P
                bias_sb = {1: b2_sb, 2: b3_sb, 3: b4_sb, 4: b5_sb}[l]
                last = p == 2
                if True:
                    if True:
                        s_t, ctn = build_s(t, [p],
                                           load=(l >= 3 and t % 4 == 1))
                        ps = pp.tile([P, 1024], F32, tag="agg",
                                     name="ps_agg")
                        if p > 0:
                            pb = hp.tile([P, 1024], FP16, tag="pb", name="pb", bufs=2)
                            nc.sync.dma_start(
                                out=pb[:, :W],
                                in_=PART[l, p - 1][t * P:(t + 1) * P, :])
                            nid = (W + 511) // 512
                            for j in range(nid):
                                w_ = min(512, W - j * 512)
                                nc.tensor.matmul(
                                    out=ps[:, j * 512: j * 512 + w_],
                                    lhsT=ident_sb[:],
                                    rhs=pb[:, j * 512: j * 512 + w_],
                                    start=True,
                                    stop=(ctn == 0 and j == nid - 1))
                            if ctn > 0:
                                gather_mm(t, p, TF[l, p][:], W, s_t, 0, ps,
                                          0, ctn, use_start=False)
                        else:
                            gather_mm(t, 0, TF[l, 0][:], W, s_t, 0, ps, 0,
                                      max(ctn, 1))
                            if ctn == 0:
                                nc.vector.memset(ps[:, :W], 0.0)
                        if not last:
                            pa = hp.tile([P, 1024], FP16, tag="pa", name="pa", bufs=2)
                            nc.scalar.activation(
                                out=pa[:, :W], in_=ps[:, :W],
                                func=mybir.ActivationFunctionType.Copy)
                            nc.sync.dma_start(
                                out=PART[l, p][t * P:(t + 1) * P, :],
                                in_=pa[:, :W])
                            return
                        hT = hp.tile([P, 1024], FP16, tag="ht", name="hT", bufs=3)
                        for fb in range(nf):
                            nc.scalar.activation(
                                out=hT[:, fb * P:(fb + 1) * P],
                                in_=ps[:, fb * P:(fb + 1) * P],
                                func=mybir.ActivationFunctionType.Relu,
                                bias=bias_sb[:, fb:fb + 1])
                        if DBG and l == 1:
                            nc.sync.dma_start(
                                out=dbg["dbg_h2"][t * P:(t + 1) * P, :],
                                in_=hT[:, :1024])
                        # dense d_{l+1}
                        if l == 1:
                            # H2[1024] @ W3 -> T3 [512]
                            t3 = hp.tile([P, 512], FP16, tag="tout", name="t3")
                            psd = pp.tile([P, 512], F32, tag="d", name="ps_d")
                            for kb in range(8):
                                nc.tensor.matmul(
                                    out=psd[:],
                                    lhsT=hT[:, kb * P:(kb + 1) * P],
                                    rhs=w3_sb[:, kb * 512:(kb + 1) * 512],
                                    start=(kb == 0), stop=(kb == 7))
                            nc.scalar.activation(
                                out=t3[:], in_=psd[:],
                                func=mybir.ActivationFunctionType.Copy)
                            if DBG:
                                nc.sync.dma_start(
                                    out=dbg["dbg_t3"][t * P:(t + 1) * P, :],
                                    in_=t3[:])
                            store_town(2, t, t3, 512)
                            if t == 15:
                                emit_ag(2, 0)
                            elif t == 31:
                                emit_ag(2, 1)
                        elif l == 2:
                            # H3[512] @ W4 -> T4 [128]
                            t4 = hp.tile([P, 128], FP16, tag="tout4", name="t4")
                            psd = pp.tile([P, 512], F32, tag="d", name="ps_d")
                            for kb in range(4):
                                nc.tensor.matmul(
                                    out=psd[:, :128],
                                    lhsT=hT[:, kb * P:(kb + 1) * P],
                                    rhs=w4_sb[:, kb * 128:(kb + 1) * 128],
                                    start=(kb == 0), stop=(kb == 3))
                            nc.scalar.activation(
                                out=t4[:], in_=psd[:, :128],
                                func=mybir.ActivationFunctionType.Copy)
                            if DBG:
                                nc.sync.dma_start(
                                    out=dbg["dbg_t4"][t * P:(t + 1) * P, :],
                                    in_=t4[:])
                            store_town(3, t, t4, 128)
                            if t == 15:
                                emit_ag(3, 0)
                            elif t == 31:
                                emit_ag(3, 1)
                        elif l == 3:
                            # H4[128] @ W5 -> T5 [128]
                            t5 = hp.tile([P, 128], FP16, tag="tout4", name="t5")
                            psd = pp.tile([P, 512], F32, tag="d", name="ps_d")
                            nc.tensor.matmul(
                                out=psd[:, :128], lhsT=hT[:, :128],
                                rhs=w5_sb[:], start=True, stop=True)
                            nc.scalar.activation(
                                out=t5[:], in_=psd[:, :128],
                                func=mybir.ActivationFunctionType.Copy)
                            if DBG:
                                nc.sync.dma_start(
                                    out=dbg["dbg_t5"][t * P:(t + 1) * P, :],
                                    in_=t5[:])
                            store_town(4, t, t5, 128)
                            if t == 15:
                                emit_ag(4, 0)
                            elif t == 31:
                                emit_ag(4, 1)
                        else:
                            # d5: out = H5 @ Wp + bp
                            psd = pp.tile([P, 512], F32, tag="d", name="ps_d")
                            nc.tensor.matmul(
                                out=psd[:, :4], lhsT=hT[:, :128], rhs=wp_sb[:],
                                start=True, stop=False)
                            nc.tensor.matmul(
                                out=psd[:, :4], lhsT=ones_sb[0:1, :],
                                rhs=bp_sb[0:1, :], start=False, stop=True)
                            osb = hp.tile([P, 4], F32, tag="fout", name="osb")
                            nc.vector.tensor_copy(out=osb[:], in_=psd[:, :4])
                            nc.sync.dma_start(
                                out=out_ext[t * P:(t + 1) * P, :],
                                in_=osb[:, :3])

            # ---------------- emission driver (software pipelining) ----------
            # phase 1 tiles 0..33, then interleave its tail with agg1-pass0
            # (whose gathers wait on the first T2 AllGather chunk).
            for t in range(34):
                phase1_tile(t)
            j = 0
            for t in range(34, NT):
                phase1_tile(t)
                while j < 2 * (t - 33) and j < NT:
                    agg_tile(1, 0, j)
                    j += 1
            emit_ag(1, 2)
            while j < NT:
                agg_tile(1, 0, j)
                j += 1
            for p in (1, 2):
                for t in range(NT):
                    agg_tile(1, p, t)
            emit_ag(2, 2)
            for l in range(2, 5):
                for p in range(3):
                    for t in range(NT):
                        agg_tile(l, p, t)
                if l < 4:
                    emit_ag(l + 1, 2)

    nc.finalize()
    return nc


# ------------------------------------------------------------------ driver
_CACHE = {}


def kernel(x, edge_index, edge_attr, W1, b1, W2, b2, W3, b3, W4, b4, W5, b5,
           Wp, bp):
    apply_tile_patch()
    import os
    from concourse.bass_utils import run_bass_kernel_spmd

    Ws = [W1, W2, W3, W4, W5]
    bs = [b1, b2, b3, b4, b5]
    in_maps, meta = preprocess(x, edge_index, edge_attr, Ws, bs, Wp, bp)

    key = (meta["K"].tobytes(), meta["nch"], meta["idxcols"])
    nc = _CACHE.get(key)
    if nc is None:
        nc = build_program(meta)
        _CACHE[key] = nc

    res = run_bass_kernel_spmd(
        nc, in_maps, core_ids=list(range(NCORE)),
        trace=bool(int(os.environ.get("TRACE", "0"))))
    if res.exec_time_ns:
        print(f"HW exec time: {res.exec_time_ns} ns")
    newslot = meta["newslot"]
    N = NCORE * PCN
    out = np.empty((N, 3), np.float32)
    for c in range(NCORE):
        rows = res.results[c]["out"]
        sel = newslot[c * PCN:(c + 1) * PCN]
        out[c * PCN:(c + 1) * PCN] = rows[sel]
    return np.ascontiguousarray(out)


# revision 47
# speedup vs baseline: 1.0124x; 1.0048x over previous
"""GCN (5-layer ColorGNN) Bass kernel for 8 TRN2 NeuronCores — v2.

Pipelined design (node-sharded SPMD, 6272 padded nodes/core = 49 tiles):

  - Full padded X (fp16, gpid layout) is staged on EVERY core: the layer-1
    aggregation gathers it directly -> no AllGather for layer 1.
  - Aggregation outputs are produced TRANSPOSED (H^T tiles: features on
    partitions, 128 dst nodes on free): out^T[f,n] = sum_e msg[e,f]*S[e,n]
    with lhsT=msg (gathered rows), rhs=S (one-hot x norm). The full
    symmetric norm dinv[src]*ew*dinv[dst] is folded into S on the host, and
    the bias is a per-partition ACT bias in the relu epilogue.
  - H^T tiles feed the next dense matmul as lhsT straight from SBUF: no
    transposes, no H round-trips through DRAM. Dense outputs T (node-major)
    go to DRAM only as AllGather input. Layer-1's dense pair runs in the
    transposed orientation (lhsT = W1 blocks) so the chain stays in H^T.
  - Each T AllGather is split in 2 node-chunks (4096+2176 rows/core,
    matching gpid regions [0,32768) and [32768,50176) which also serve as
    the int16 gather-index regions). Chunk0 fires when dense tiles 0..31
    are stored; the next layer's aggregation runs in 2 passes (region A
    then region B) so its region-A gathers need only chunk0 -> chunk1's
    transfer hides under the region-A gather. Pass A's PSUM is staged to
    DRAM as an fp16 partial and re-added in pass B via an identity matmul.
  - Host preprocessing: edges bucketed per (core=dst core, dst tile, src
    region), padded to 128-multiples of the cross-core max so all 8 cores
    run one SPMD program.
"""

import numpy as np
import concourse.bass as bass
import concourse.mybir as mybir
import concourse.tile as tile

FP16 = mybir.dt.float16
F32 = mybir.dt.float32
I16 = mybir.dt.int16

P = 128
NCORE = 8
PCN = 6250            # real nodes per core
NPC = 6272            # padded nodes per core (49 tiles)
NT = 49
# AllGather chunks per core (tile-aligned): tiles 0-15 / 16-31 / 32-48
CH = [0, 2048, 4096, 6272]       # per-core row boundaries
GB = [0, 16384, 32768, 50176]    # global gpid base of each chunk
REG0 = 32768          # gpid boundary of gather region 0 (chunks 0a+0b)
NPT = NCORE * NPC     # 50176 padded total nodes
T_SPLIT = 32          # first tile of gather-region 1
GMAX = 8              # chunks per gather call (1024 idx ucode cap)
SBATCH = 16           # chunks per batched S-build DVE op

FEAT = 512
WG = [512, 1024, 512, 128, 128]   # gather/agg width per layer l = 0..4
# dense widths: d0: 512->2048(T out), d1: 2048->1024, d2: 1024->512,
#               d3: 512->128, d4: 128->128, d5: 128->4


# ---------------------------------------------------------------- tile patch
def apply_tile_patch():
    """This walrus build allows only 1 sync-wait per Drain; split the tail
    drain's waits across a chain of drains."""
    import bass_rust

    def _drain_and_barrier_split(self, tick_clock, wait_clock):
        from bass_rust import ScopedClock
        drain_inst = self.nc.sync.drain()
        wait_clock.add_sem_waits(
            drain_inst.ins, ScopedClock({None: tick_clock.global_clock})
        )
        si = drain_inst.ins.sync_info
        waits = list(si.on_wait) if si is not None else []
        if len(waits) > 1:
            si.on_wait = [waits[0]]
            for w in waits[1:]:
                extra = self.nc.sync.drain()
                if extra.ins.sync_info is None:
                    extra.ins.sync_info = bass_rust.SyncInfo(
                        on_wait=[w], on_update=[])
                else:
                    extra.ins.sync_info.on_wait = [w]
        self.nc.all_engine_barrier()
        popped = self.nc._tile_sem_poison_stack.pop()
        assert popped is self._sem_poison
        self.nc.clear_and_free_semaphores(list(self.sems.allocated().values()))
        self.nc.all_engine_barrier()

    tile.TileContext._drain_and_barrier = _drain_and_barrier_split


def _gpid(core, slot):
    """Chunk-major padded global id (vectorized over 3 AG chunks)."""
    out = np.empty_like(np.broadcast_arrays(core, slot)[1])
    core = np.asarray(core)
    slot = np.asarray(slot)
    for j in range(3):
        m = (slot >= CH[j]) & (slot < CH[j + 1])
        w = CH[j + 1] - CH[j]
        out[m] = GB[j] + core[m] * w + (slot[m] - CH[j])
    return out


# ------------------------------------------------------------- preprocess
def preprocess(x, edge_index, edge_attr, Ws, bs, Wp, bp):
    """Host-side: normalization, edge sharding/sorting/packing, weight packs.
    Returns (in_maps, meta)."""
    N = NCORE * PCN
    src = np.asarray(edge_index[0], dtype=np.int64)
    dst = np.asarray(edge_index[1], dtype=np.int64)
    ew = np.asarray(edge_attr, dtype=np.float32)
    loop = np.arange(N, dtype=np.int64)
    src2 = np.concatenate([src, loop])
    dst2 = np.concatenate([dst, loop])
    ew2 = np.concatenate([ew, np.ones(N, np.float32)])

    deg = np.bincount(dst2, weights=ew2.astype(np.float64), minlength=N)
    deg = deg.astype(np.float32)
    dinv = np.where(deg > 0, 1.0 / np.sqrt(deg), 0.0).astype(np.float32)
    normf = (dinv[src2] * ew2 * dinv[dst2]).astype(np.float32)  # full norm

    score = src2 // PCN

    # ---- degree-balanced node->tile packing (within each (core, chunk)):
    # re-bin nodes into tiles so per-(tile, region) edge counts are nearly
    # equal across cores and land just under multiples of 128, minimizing
    # SPMD padding. A node keeps its (core, chunk), so src regions are
    # unchanged by the permutation.
    allslot = np.arange(N, dtype=np.int64) % PCN
    chunk_of_node = np.digitize(allslot, CH[1:3])          # [N] in {0,1,2}
    src_chunk = chunk_of_node[src2]
    dvec = np.zeros((N, 3), np.int64)
    np.add.at(dvec, (dst2, src_chunk), 1)

    # per-(core, chunk, r) totals -> global per-(tile-in-chunk, r) caps
    core_all = np.arange(N, dtype=np.int64) // PCN
    Tcr = np.zeros((NCORE, 3, 3), np.int64)
    for r in range(3):
        np.add.at(Tcr, (core_all, chunk_of_node, r), dvec[:, r])
    newslot = np.zeros(N, np.int64)
    for j in range(3):
        nt_j = (CH[j + 1] - CH[j]) // P
        maxT = Tcr[:, j, :].max(axis=0)                    # [3]
        caps = np.zeros((nt_j, 3), np.int64)
        for r in range(3):
            m_tot = int(np.ceil((maxT[r] + 6 * nt_j) / P))
            base, extra = divmod(m_tot, nt_j)
            caps[:, r] = P * base
            caps[:extra, r] += P
        for c in range(NCORE):
            vs = np.where((core_all == c) & (chunk_of_node == j))[0]
            dv = dvec[vs]
            order_v = np.argsort(-dv.sum(axis=1), kind="stable")
            used = np.zeros((nt_j, 3), np.float64)
            cnt = np.zeros(nt_j, np.int64)
            pos = np.zeros(nt_j, np.int64)
            for vi in order_v:
                nd = dv[vi]
                ok = cnt < P
                over = np.maximum(used + nd - caps, 0).sum(axis=1)
                frac = ((used + nd) / caps).max(axis=1)
                tscore = over * 1e9 + frac
                tscore[~ok] = np.inf
                ti = int(tscore.argmin())
                used[ti] += nd
                cnt[ti] += 1
                newslot[vs[vi]] = CH[j] + ti * P + pos[ti]
                pos[ti] += 1

    core_of = dst2 // PCN
    slot = newslot[dst2]
    tile_of = slot // P
    slot_in = slot % P
    sslot = newslot[src2]
    gpid = _gpid(score, sslot)
    region_of = np.digitize(gpid, GB[1:3]).astype(np.int64)

    NR = 3
    counts = np.zeros((NCORE, NT, NR), np.int64)
    np.add.at(counts, (core_of, tile_of, region_of), 1)
    kmax = counts.max(axis=0)                      # [NT, NR]
    K = ((kmax + P - 1) // P) * P
    K[kmax == 0] = 0

    order = np.lexsort((region_of, tile_of, core_of))
    so_gpid = gpid[order]
    so_norm = normf[order]
    so_slot = slot_in[order]
    so_core = core_of[order]
    so_tile = tile_of[order]
    so_reg = region_of[order]

    icol = np.zeros((NT, NR), np.int64)
    cbase = np.zeros((NT, NR), np.int64)
    ic = cc = 0
    for t in range(NT):
        for r in range(NR):
            icol[t, r] = ic
            cbase[t, r] = cc
            ic += K[t, r] // 16
            cc += K[t, r] // P
    idxcols, nch = ic, cc

    # full padded X in gpid layout (shared by all cores)
    x_np = np.asarray(x, dtype=np.float32)
    xf = np.zeros((NPT, FEAT), np.float16)
    nodes = np.arange(N, dtype=np.int64)
    xf[_gpid(nodes // PCN, newslot)] = x_np.astype(np.float16)

    # weight packs
    w_list = [np.asarray(w, np.float32) for w in Ws] + [np.asarray(Wp, np.float32)]
    b_list = [np.asarray(b, np.float32) for b in bs] + [np.asarray(bp, np.float32)]
    # W1 [512,2048] as lhsT blocks (kb in 4, mb in 16): col (kb*16+mb)*128
    W1 = w_list[0]
    w1p = np.zeros((P, 4 * 16 * P), np.float16)
    for kb in range(4):
        for mb in range(16):
            w1p[:, (kb * 16 + mb) * P:(kb * 16 + mb + 1) * P] = (
                W1[kb * P:(kb + 1) * P, mb * P:(mb + 1) * P].astype(np.float16))
    # W2..W5, Wp as rhs blocks (kb-major): [128, nk*Md]
    def rhs_pack(Wr, Kd, Md):
        wp_ = np.zeros((Kd, Md), np.float32)
        wp_[: Wr.shape[0], : Wr.shape[1]] = Wr
        nk = Kd // P
        out = np.zeros((P, nk * Md), np.float16)
        for kb in range(nk):
            out[:, kb * Md:(kb + 1) * Md] = wp_[kb * P:(kb + 1) * P].astype(np.float16)
        return out

    w2p = rhs_pack(w_list[1], 2048, 1024)
    w3p = rhs_pack(w_list[2], 1024, 512)
    w4p = rhs_pack(w_list[3], 512, 128)
    w5p = rhs_pack(w_list[4], 128, 128)
    wpp = rhs_pack(w_list[5], 128, 4)

    # biases as per-partition columns [128, nblk]
    def bias_cols(b, width):
        bb = np.zeros(width, np.float32)
        bb[: b.shape[0]] = b
        return bb.reshape(width // P, P).T.astype(np.float16).copy()

    b1c = bias_cols(b_list[0], 2048)   # d0 epilogue (relu)
    b2c = bias_cols(b_list[1], 1024)   # agg1 epilogue
    b3c = bias_cols(b_list[2], 512)    # agg2
    b4c = bias_cols(b_list[3], 128)    # agg3
    b5c = bias_cols(b_list[4], 128)    # agg4
    bpr = np.zeros((1, 4), np.float16)
    bpr[0, :3] = b_list[5].astype(np.float16)

    iota = np.tile(np.arange(P, dtype=np.float16), (P, 1))
    ident = np.eye(P, dtype=np.float16)
    ones1 = np.ones((1, P), np.float16)

    core_starts = np.searchsorted(so_core, np.arange(NCORE + 1))
    in_maps = []
    for c in range(NCORE):
        lo, hi = core_starts[c], core_starts[c + 1]
        ct, cr = so_tile[lo:hi], so_reg[lo:hi]
        cg, cn, cs = so_gpid[lo:hi], so_norm[lo:hi], so_slot[lo:hi]
        idx16 = np.zeros((16, idxcols), np.int16)
        slotp = np.zeros((P, nch), np.float16)
        normp = np.zeros((P, nch), np.float16)
        pos = 0
        for t in range(NT):
            for r in range(NR):
                k = int(K[t, r])
                if k == 0:
                    continue
                n_e = int(counts[c, t, r])
                seg = slice(pos, pos + n_e)
                assert np.all(ct[seg] == t) and np.all(cr[seg] == r), (c, t, r)
                reg_lo = GB[r]
                arr = np.zeros(k, np.int64)
                arr[:n_e] = cg[seg] - reg_lo
                assert 0 <= arr.min(initial=0) and arr.max(initial=0) < 32768
                idx16[:, int(icol[t, r]): int(icol[t, r]) + k // 16] = (
                    arr.reshape(k // 16, 16).T.astype(np.int16))
                sl = np.zeros(k, np.float32)
                sl[:n_e] = cs[seg]
                nm = np.zeros(k, np.float32)
                nm[:n_e] = cn[seg]
                cb = int(cbase[t, r])
                slotp[:, cb: cb + k // P] = (
                    sl.reshape(k // P, P).T.astype(np.float16))
                normp[:, cb: cb + k // P] = (
                    nm.reshape(k // P, P).T.astype(np.float16))
                pos += n_e
        assert pos == hi - lo

        # full S (one-hot x norm), for layers that load S instead of
        # building it on DVE
        oh = (slotp[:, :, None] == np.arange(P, dtype=np.float16)[None, None, :])
        sfull = (oh * normp[:, :, None]).astype(np.float16).reshape(P, nch * P)
        m = {
            "xf": xf,
            "idx16": np.tile(idx16, (8, 1)),
            "slotp": slotp,
            "normp": normp,
            "sfull": sfull,
            "iota": iota,
            "ident": ident,
            "ones1": ones1,
            "w1": w1p, "w2": w2p, "w3": w3p, "w4": w4p, "w5": w5p, "wp": wpp,
            "b1": b1c, "b2": b2c, "b3": b3c, "b4": b4c, "b5": b5c, "bp": bpr,
        }
        in_maps.append(m)

    meta = dict(K=K, icol=icol, cbase=cbase, idxcols=idxcols, nch=nch,
                newslot=newslot)
    return in_maps, meta


# ---------------------------------------------------------------- program
def _bc3(ap, ncols, inner=P, mode="col"):
    base = ap.ap
    if mode == "col":
        return bass.AP(ap.tensor, ap.offset, [base[0], [1, ncols], [0, inner]])
    else:
        return bass.AP(ap.tensor, ap.offset, [base[0], [0, ncols], base[1]])


def _3d(ap, ncols, inner=P):
    return bass.AP(ap.tensor, ap.offset, [ap.ap[0], [inner, ncols], [1, inner]])


def build_program(meta):
    import concourse.bacc as bacc
    nc = bacc.Bacc("TRN2", num_swdge_queues=4)
    K, icol, cbase = meta["K"], meta["icol"], meta["cbase"]
    idxcols, nch = meta["idxcols"], meta["nch"]
    rg = [list(range(NCORE))]

    # ---------------- params
    pr = {}
    def par(name, shape, dt):
        pr[name] = nc.declare_dram_parameter(name, shape, dt, isOutput=False)
    par("xf", [NPT, FEAT], FP16)
    par("idx16", [P, idxcols], I16)
    par("slotp", [P, nch], FP16)
    par("normp", [P, nch], FP16)
    par("sfull", [P, nch * P], FP16)
    par("iota", [P, P], FP16)
    par("ident", [P, P], FP16)
    par("ones1", [1, P], FP16)
    par("w1", [P, 4 * 16 * P], FP16)
    par("w2", [P, 16 * 1024], FP16)
    par("w3", [P, 8 * 512], FP16)
    par("w4", [P, 4 * 128], FP16)
    par("w5", [P, 128], FP16)
    par("wp", [P, 4], FP16)
    par("b1", [P, 16], FP16)
    par("b2", [P, 8], FP16)
    par("b3", [P, 4], FP16)
    par("b4", [P, 1], FP16)
    par("b5", [P, 1], FP16)
    par("bp", [1, 4], FP16)
    out_ext = nc.declare_dram_parameter("out", [NPC, 3], F32, isOutput=True)
    import os
    DBG = bool(int(os.environ.get("KDBG", "0")))
    dbg = {}
    if DBG:
        for nm, shape in [("dbg_g1", [NT * P, 512]), ("dbg_h1", [NT * P, 2048]),
                          ("dbg_t2", [NPC, 1024]), ("dbg_h2", [NT * P, 1024]),
                          ("dbg_t3", [NPC, 512]), ("dbg_t4", [NPC, 128]),
                          ("dbg_t5", [NPC, 128])]:
            dbg[nm] = nc.declare_dram_parameter(nm, shape, FP16, isOutput=True)

    # ---------------- internal DRAM
    # layers l=1..4 aggregate T_{l+1}; width WG[l]
    town = {}   # (l, chunk j) -> per-core town tensor
    for l in range(1, 5):
        for j in range(3):
            town[l, j] = nc.dram_tensor(
                f"tn{l}_{j}", [CH[j + 1] - CH[j], WG[l]], FP16)
    TF = {}
    for l in range(1, 5):
        for j in range(3):
            TF[l, j] = nc.dram_tensor(
                f"tf{l}_{j}", [(GB[j + 1] - GB[j]), WG[l]], FP16,
                addr_space="Shared")
    PART = {(l, p): nc.dram_tensor(f"part{l}_{p}", [NT * P, WG[l]], FP16)
            for l in range(1, 5) for p in range(2)}

    with tile.TileContext(nc) as tc:
        import contextlib
        with contextlib.ExitStack() as ctx:
            cpool = ctx.enter_context(tc.tile_pool(name="const", bufs=1))
            msgp = ctx.enter_context(tc.tile_pool(name="msg", bufs=7))
            spool = ctx.enter_context(tc.tile_pool(name="sb", bufs=3))
            pp = ctx.enter_context(tc.tile_pool(name="ps", bufs=2, space="PSUM"))
            hp = ctx.enter_context(tc.tile_pool(name="hh", bufs=2))

            # ---- resident constants
            def cload(name, shape, dt):
                t_ = cpool.tile(shape, dt, tag=name, name=name)
                nc.sync.dma_start(out=t_[:], in_=pr[name][:])
                return t_
            idx_sb = cload("idx16", [P, idxcols], I16)
            slot_sb = cload("slotp", [P, nch], FP16)
            norm_sb = cload("normp", [P, nch], FP16)
            iota_sb = cload("iota", [P, P], FP16)
            ident_sb = cload("ident", [P, P], FP16)
            ones_sb = cload("ones1", [1, P], FP16)
            w1_sb = cload("w1", [P, 4 * 16 * P], FP16)
            w2_sb = cload("w2", [P, 16 * 1024], FP16)
            w3_sb = cload("w3", [P, 8 * 512], FP16)
            w4_sb = cload("w4", [P, 4 * 128], FP16)
            w5_sb = cload("w5", [P, 128], FP16)
            wp_sb = cload("wp", [P, 4], FP16)
            b1_sb = cload("b1", [P, 16], FP16)
            b2_sb = cload("b2", [P, 8], FP16)
            b3_sb = cload("b3", [P, 4], FP16)
            b4_sb = cload("b4", [P, 1], FP16)
            b5_sb = cload("b5", [P, 1], FP16)
            bp_sb = cload("bp", [1, 4], FP16)

            qn = [0]

            def build_s(t, r_list, load=False):
                """One S tile covering the chunks of (tile t, regions r_list)
                (contiguous in cbase layout). load=True DMAs the precomputed
                S from DRAM instead of building it on DVE."""
                c_lo = int(cbase[t, r_list[0]])
                ctn = sum(int(K[t, r]) // P for r in r_list)
                s_t = spool.tile([P, max(ctn, 1) * P], FP16, tag="s", name="s_t")
                if load:
                    if ctn > 0:
                        nc.sync.dma_start(
                            out=s_t[:, : ctn * P],
                            in_=pr["sfull"][:, c_lo * P:(c_lo + ctn) * P])
                    return s_t, ctn
                for b0 in range(0, ctn, SBATCH):
                    nb = min(SBATCH, ctn - b0)
                    cb0 = c_lo + b0
                    o3 = _3d(s_t[:, b0 * P:(b0 + nb) * P], nb)
                    nc.vector.tensor_tensor(
                        out=o3,
                        in0=_bc3(slot_sb[:, cb0:cb0 + nb], nb, mode="col"),
                        in1=_bc3(iota_sb[:], nb, mode="mat"),
                        op=mybir.AluOpType.is_equal)
                    nc.vector.tensor_tensor(
                        out=o3, in0=o3,
                        in1=_bc3(norm_sb[:, cb0:cb0 + nb], nb, mode="col"),
                        op=mybir.AluOpType.mult)
                return s_t, ctn

            def gather_mm(t, r, src_dram, W, s_t, s_coff, ps, mmcnt, mmtot,
                          use_start=True):
                """Gathers + aggregation matmuls for (tile t, region r).
                s_coff: chunk offset of this region within s_t.
                mmcnt: chunks already accumulated into ps; mmtot: total
                expected (stop flags on the last). use_start=False when the
                psum was already initialized (identity re-add). Returns new
                mmcnt."""
                k = int(K[t, r])
                if k == 0:
                    return mmcnt
                nf = W // P
                gmax = GMAX if W <= 512 else (4096 // W)  # cap msg at 8KB/part
                pos = 0
                while pos < k:
                    ks = min(gmax * P, k - pos)
                    ic = int(icol[t, r]) + pos // 16
                    msg = msgp.tile([P, 4096], FP16, tag="msg", name="msg")
                    nc.gpsimd.dma_gather(
                        out_ap=_3d(msg[:, : (ks // P) * W], ks // P, inner=W),
                        in_ap=src_dram,
                        idxs_ap=idx_sb[:, ic: ic + ks // 16],
                        num_idxs=ks,
                        num_idxs_reg=ks,
                        elem_size=W,
                        elem_step=W,
                        queue_num=qn[0],
                    )
                    qn[0] = (qn[0] + 1) % 4
                    for ci in range(ks // P):
                        cglob = s_coff + pos // P + ci
                        for fb in range(nf):
                            # start only on the first write to each 2KB PSUM
                            # zero region (512 f32 cols = 4 fb slices)
                            nc.tensor.matmul(
                                out=ps[:, fb * P:(fb + 1) * P],
                                lhsT=msg[:, ci * W + fb * P: ci * W + (fb + 1) * P],
                                rhs=s_t[:, cglob * P:(cglob + 1) * P],
                                start=(use_start and mmcnt == 0
                                       and fb % 4 == 0),
                                stop=(mmcnt == mmtot - 1))
                        mmcnt += 1
                    pos += ks
                return mmcnt

            def store_town(l, t, t_sb, W):
                j = 0 if t < 16 else (1 if t < 32 else 2)
                r0 = t * P - CH[j]
                nc.sync.dma_start(
                    out=town[l, j][r0:r0 + P, :], in_=t_sb[:, :W])

            def emit_ag(l, j):
                nc.gpsimd.collective_compute(
                    "AllGather", mybir.AluOpType.bypass, replica_groups=rg,
                    ins=[town[l, j][:]], outs=[TF[l, j][:]])

            # ================= phase 1: agg0(X) + d0 + d1 -> T2 =================
            def phase1_tile(t):
                s_t, ctn = build_s(t, [0, 1, 2])
                ps = pp.tile([P, 1024], F32, tag="agg", name="ps_agg")
                mm = 0
                coff = 0
                for r in range(3):
                    mm = gather_mm(t, r, pr["xf"][GB[r]:GB[r + 1], :], 512,
                                   s_t, coff, ps, mm, ctn)
                    coff += int(K[t, r]) // P
                assert mm == ctn and ctn > 0
                g1t = hp.tile([P, 512], FP16, tag="g1t", name="g1t")
                nc.scalar.activation(
                    out=g1t[:], in_=ps[:, :512],
                    func=mybir.ActivationFunctionType.Copy)
                if DBG:
                    nc.sync.dma_start(out=dbg["dbg_g1"][t * P:(t + 1) * P, :],
                                      in_=g1t[:])
                # d0: H1^T = relu(W1^T-blocks @ G1^T + b1), 4 quarters
                h1t = hp.tile([P, 2048], FP16, tag="h1t", name="h1t")
                for q in range(4):
                    ps0 = pp.tile([P, 512], F32, tag="d0", name="ps_d0")
                    for mi in range(4):
                        mb = q * 4 + mi
                        for kb in range(4):
                            nc.tensor.matmul(
                                out=ps0[:, mi * P:(mi + 1) * P],
                                lhsT=w1_sb[:, (kb * 16 + mb) * P:(kb * 16 + mb + 1) * P],
                                rhs=g1t[:, kb * P:(kb + 1) * P],
                                start=(kb == 0 and mi == 0),
                                stop=(kb == 3))
                    for mi in range(4):
                        mb = q * 4 + mi
                        nc.scalar.activation(
                            out=h1t[:, mb * P:(mb + 1) * P],
                            in_=ps0[:, mi * P:(mi + 1) * P],
                            func=mybir.ActivationFunctionType.Relu,
                            bias=b1_sb[:, mb:mb + 1])
                # d1: T2 = H1 @ W2 (normal orientation), 2 halves of 512
                t2sb = hp.tile([P, 1024], FP16, tag="tout", name="t2sb")
                for h in range(2):
                    psd = pp.tile([P, 512], F32, tag="d", name="ps_d")
                    for kb in range(16):
                        nc.tensor.matmul(
                            out=psd[:],
                            lhsT=h1t[:, kb * P:(kb + 1) * P],
                            rhs=w2_sb[:, kb * 1024 + h * 512: kb * 1024 + h * 512 + 512],
                            start=(kb == 0), stop=(kb == 15))
                    nc.scalar.activation(
                        out=t2sb[:, h * 512:(h + 1) * 512], in_=psd[:],
                        func=mybir.ActivationFunctionType.Copy)
                if DBG:
                    nc.sync.dma_start(out=dbg["dbg_h1"][t * P:(t + 1) * P, :],
                                      in_=h1t[:])
                    nc.sync.dma_start(out=dbg["dbg_t2"][t * P:(t + 1) * P, :],
                                      in_=t2sb[:])
                store_town(1, t, t2sb, 1024)
                if t == 15:
                    emit_ag(1, 0)
                elif t == 31:
                    emit_ag(1, 1)

            # ========= layers l=1..4: agg_l (3 passes, 1 per region) + dense ====
            # agg_l consumes TF[l,*] (width WG[l]), produces H^{l+1,T}; dense
            # d_{l+1} produces T_{l+2} (towns l+1) or the final output. Passes
            # 0/1 stage the PSUM to DRAM as fp16 partials; passes 1/2 re-add
            # them via an identity matmul.
            def agg_tile(l, p, t):
                W = WG[l]
                nf = W // P
                bias_sb = {1: b2_sb, 2: b3_sb, 3: b4_sb, 4: b5_sb}[l]
                last = p == 2
                if True:
                    if True:
                        s_t, ctn = build_s(t, [p],
                                           load=(l >= 3 and t % 4 == 1))
                        ps = pp.tile([P, 1024], F32, tag="agg",
                                     name="ps_agg")
                        if p > 0:
                            pb = hp.tile([P, 1024], FP16, tag="pb", name="pb", bufs=2)
                            nc.sync.dma_start(
                                out=pb[:, :W],
                                in_=PART[l, p - 1][t * P:(t + 1) * P, :])
                            nid = (W + 511) // 512
                            for j in range(nid):
                                w_ = min(512, W - j * 512)
                                nc.tensor.matmul(
                                    out=ps[:, j * 512: j * 512 + w_],
                                    lhsT=ident_sb[:],
                                    rhs=pb[:, j * 512: j * 512 + w_],
                                    start=True,
                                    stop=(ctn == 0 and j == nid - 1))
                            if ctn > 0:
                                gather_mm(t, p, TF[l, p][:], W, s_t, 0, ps,
                                          0, ctn, use_start=False)
                        else:
                            gather_mm(t, 0, TF[l, 0][:], W, s_t, 0, ps, 0,
                                      max(ctn, 1))
                            if ctn == 0:
                                nc.vector.memset(ps[:, :W], 0.0)
                        if not last:
                            pa = hp.tile([P, 1024], FP16, tag="pa", name="pa", bufs=2)
                            nc.scalar.activation(
                                out=pa[:, :W], in_=ps[:, :W],
                                func=mybir.ActivationFunctionType.Copy)
                            nc.sync.dma_start(
                                out=PART[l, p][t * P:(t + 1) * P, :],
                                in_=pa[:, :W])
                            return
                        hT = hp.tile([P, 1024], FP16, tag="ht", name="hT", bufs=3)
                        for fb in range(nf):
                            nc.scalar.activation(
                                out=hT[:, fb * P:(fb + 1) * P],
                                in_=ps[:, fb * P:(fb + 1) * P],
                                func=mybir.ActivationFunctionType.Relu,
                                bias=bias_sb[:, fb:fb + 1])
                        if DBG and l == 1:
                            nc.sync.dma_start(
                                out=dbg["dbg_h2"][t * P:(t + 1) * P, :],
                                in_=hT[:, :1024])
                        # dense d_{l+1}
                        if l == 1:
                            # H2[1024] @ W3 -> T3 [512]
                            t3 = hp.tile([P, 512], FP16, tag="tout", name="t3")
                            psd = pp.tile([P, 512], F32, tag="d", name="ps_d")
                            for kb in range(8):
                                nc.tensor.matmul(
                                    out=psd[:],
                                    lhsT=hT[:, kb * P:(kb + 1) * P],
                                    rhs=w3_sb[:, kb * 512:(kb + 1) * 512],
                                    start=(kb == 0), stop=(kb == 7))
                            nc.scalar.activation(
                                out=t3[:], in_=psd[:],
                                func=mybir.ActivationFunctionType.Copy)
                            if DBG:
                                nc.sync.dma_start(
                                    out=dbg["dbg_t3"][t * P:(t + 1) * P, :],
                                    in_=t3[:])
                            store_town(2, t, t3, 512)
                            if t == 15:
                                emit_ag(2, 0)
                            elif t == 31:
                                emit_ag(2, 1)
                        elif l == 2:
                            # H3[512] @ W4 -> T4 [128]
                            t4 = hp.tile([P, 128], FP16, tag="tout4", name="t4")
                            psd = pp.tile([P, 512], F32, tag="d", name="ps_d")
                            for kb in range(4):
                                nc.tensor.matmul(
                                    out=psd[:, :128],
                                    lhsT=hT[:, kb * P:(kb + 1) * P],
                                    rhs=w4_sb[:, kb * 128:(kb + 1) * 128],
                                    start=(kb == 0), stop=(kb == 3))
                            nc.scalar.activation(
                                out=t4[:], in_=psd[:, :128],
                                func=mybir.ActivationFunctionType.Copy)
                            if DBG:
                                nc.sync.dma_start(
                                    out=dbg["dbg_t4"][t * P:(t + 1) * P, :],
                                    in_=t4[:])
                            store_town(3, t, t4, 128)
                            if t == 15:
                                emit_ag(3, 0)
                            elif t == 31:
                                emit_ag(3, 1)
                        elif l == 3:
                            # H4[128] @ W5 -> T5 [128]
                            t5 = hp.tile([P, 128], FP16, tag="tout4", name="t5")
                            psd = pp.tile([P, 512], F32, tag="d", name="ps_d")
                            nc.tensor.matmul(
                                out=psd[:, :128], lhsT=hT[:, :128],
                                rhs=w5_sb[:], start=True, stop=True)
                            nc.scalar.activation(
                                out=t5[:], in_=psd[:, :128],
                                func=mybir.ActivationFunctionType.Copy)
                            if DBG:
                                nc.sync.dma_start(
                                    out=dbg["dbg_t5"][t * P:(t + 1) * P, :],
                                    in_=t5[:])
                            store_town(4, t, t5, 128)
                            if t == 15:
                                emit_ag(4, 0)
                            elif t == 31:
                                emit_ag(4, 1)
                        else:
                            # d5: out = H5 @ Wp + bp
                            psd = pp.tile([P, 512], F32, tag="d", name="ps_d")
                            nc.tensor.matmul(
                                out=psd[:, :4], lhsT=hT[:, :128], rhs=wp_sb[:],
                                start=True, stop=False)
                            nc.tensor.matmul(
                                out=psd[:, :4], lhsT=ones_sb[0:1, :],
                                rhs=bp_sb[0:1, :], start=False, stop=True)
                            osb = hp.tile([P, 4], F32, tag="fout", name="osb")
                            nc.vector.tensor_copy(out=osb[:], in_=psd[:, :4])
                            nc.sync.dma_start(
                                out=out_ext[t * P:(t + 1) * P, :],
                                in_=osb[:, :3])

            # ---------------- emission driver (software pipelining) ----------
            # phase 1 tiles 0..33, then interleave its tail with agg1-pass0
            # (whose gathers wait on the first T2 AllGather chunk).
            for t in range(34):
                phase1_tile(t)
            j = 0
            for t in range(34, NT):
                phase1_tile(t)
                while j < 2 * (t - 33) and j < NT:
                    agg_tile(1, 0, j)
                    j += 1
            emit_ag(1, 2)
            while j < NT:
                agg_tile(1, 0, j)
                j += 1
            for p in (1, 2):
                for t in range(NT):
                    agg_tile(1, p, t)
            emit_ag(2, 2)
            for l in range(2, 5):
                for p in range(3):
                    for t in range(NT):
                        agg_tile(l, p, t)
                if l < 4:
                    emit_ag(l + 1, 2)

    nc.finalize()
    return nc


# ------------------------------------------------------------------ driver
_CACHE = {}


def kernel(x, edge_index, edge_attr, W1, b1, W2, b2, W3, b3, W4, b4, W5, b5,
           Wp, bp):
    apply_tile_patch()
    import os
    from concourse.bass_utils import run_bass_kernel_spmd

    Ws = [W1, W2, W3, W4, W5]
    bs = [b1, b2, b3, b4, b5]
    in_maps, meta = preprocess(x, edge_index, edge_attr, Ws, bs, Wp, bp)

    key = (meta["K"].tobytes(), meta["nch"], meta["idxcols"])
    nc = _CACHE.get(key)
    if nc is None:
        nc = build_program(meta)
        _CACHE[key] = nc

    res = run_bass_kernel_spmd(
        nc, in_maps, core_ids=list(range(NCORE)),
        trace=bool(int(os.environ.get("TRACE", "0"))))
    if res.exec_time_ns:
        print(f"HW exec time: {res.exec_time_ns} ns")
    newslot = meta["newslot"]
    N = NCORE * PCN
    out = np.empty((N, 3), np.float32)
    for c in range(NCORE):
        rows = res.results[c]["out"]
        sel = newslot[c * PCN:(c + 1) * PCN]
        out[c * PCN:(c + 1) * PCN] = rows[sel]
    return np.ascontiguousarray(out)
